# revision 1
# baseline (speedup 1.0000x reference)
"""DGCNN forward on 8 Trainium2 NeuronCores, data-parallel over batch.

Contract: kernel(**inputs) takes the FULL (unsharded) inputs from
reference.setup_inputs() and returns the FULL (32, 40) output.

Algorithm (exact, fp32):
  EdgeConv(x)_i = max_{j in knn20(i)} relu(bn(W @ [x_j - x_i; x_i]))
 decomposes (relu/max commute, bn is affine) into
  u_j = s*(wA @ x_j);  v_i = s*((wB-wA) @ x_i) + b
  out_i = relu( max_{j in knn20(i)} u_j  +  v_i )
 so each layer is: pairwise-distance matmul (PE) -> exact top-20 row
 selection (DVE max8/match_replace/max_index) -> gather u rows by index
 (GPSIMD ap_gather) -> windowed max (DVE reduce) -> +v, relu (ACT).
"""

import numpy as np

B, N, K = 32, 1024, 20
EPS = 1e-5
NCORES = 8
BPC = B // NCORES          # batches per core
NEG = -1e30

_CACHE = {}


# ---------------------------------------------------------------- weight prep
def _prep_weights(inp):
    """Fold BN into the edge-conv and MLP weights (numpy, host-side)."""
    w = {}
    couts = [64, 64, 64, 128]
    cins = [3, 64, 64, 64]
    for l in range(4):
        wl = inp[f'w{l+1}']            # (Cout, 2C)
        g = inp[f'g{l+1}']
        b = inp[f'b{l+1}']
        C = cins[l]
        s = g / np.sqrt(1.0 + EPS)
        wA = wl[:, :C]                  # acts on (x_j - x_i)
        wB = wl[:, C:]                  # acts on x_i
        Wu = (s[:, None] * wA).T.astype(np.float32)           # (C, Cout)
        Wv = (s[:, None] * (wB - wA)).T.astype(np.float32)    # (C, Cout)
        cout = couts[l]
        if l < 3:
            # batch-pair packing: [Wu | 0] and [0 | Wu], (C, 128)
            zu = np.zeros((C, 64), np.float32)
            w[f'wu{l}a'] = np.concatenate([Wu, zu], 1)
            w[f'wu{l}b'] = np.concatenate([zu, Wu], 1)
            w[f'wv{l}a'] = np.concatenate([Wv, zu], 1)
            w[f'wv{l}b'] = np.concatenate([zu, Wv], 1)
            w[f'bv{l}'] = np.concatenate([b, b]).reshape(128, 1).astype(np.float32)
        else:
            w[f'wu{l}'] = Wu            # (64, 128)
            w[f'wv{l}'] = Wv
            w[f'bv{l}'] = b.reshape(128, 1).astype(np.float32)
    s5 = inp['g5'] / np.sqrt(1.0 + EPS)
    w['w1t'] = (s5[:, None] * inp['lw1']).T.astype(np.float32)      # (320, 1024)
    w['b1'] = (s5 * inp['lb1'] + inp['b5']).reshape(8, 128).T.astype(np.float32).copy()  # (128, 8)
    s6 = inp['g6'] / np.sqrt(1.0 + EPS)
    w['w2t'] = (s6[:, None] * inp['lw2']).T.astype(np.float32)      # (1024, 512)
    w['b2'] = (s6 * inp['lb2'] + inp['b6']).reshape(4, 128).T.astype(np.float32).copy()  # (128, 4)
    w['w3t'] = inp['lw3'].T.astype(np.float32)                      # (512, 40)
    w['b3'] = inp['lb3'].reshape(40, 1).astype(np.float32)
    return w


# ---------------------------------------------------------------- bass program
def _build_program(n_layers=4, with_mlp=True):
    import concourse.bass as bass
    import concourse.bacc as bacc
    import concourse.mybir as mybir
    from concourse.tile import TileContext

    f32 = mybir.dt.float32
    u16 = mybir.dt.uint16
    i16 = mybir.dt.int16
    AF = mybir.ActivationFunctionType
    AX = mybir.AxisListType

    nc = bacc.Bacc("TRN2")

    # ---- DRAM tensors (per-core inputs) ----
    xT = nc.dram_tensor("xT", [BPC, 3, N], f32, kind="ExternalInput").ap()
    cins = [3, 64, 64, 64]
    couts = [64, 64, 64, 128]
    wt = {}
    for l in range(3):
        for nm in ('wua', 'wub', 'wva', 'wvb'):
            key = f'{nm[:2]}{l}{nm[2]}'
            wt[key] = nc.dram_tensor(key, [cins[l], 128], f32, kind="ExternalInput").ap()
        wt[f'bv{l}'] = nc.dram_tensor(f'bv{l}', [128, 1], f32, kind="ExternalInput").ap()
    wt['wu3'] = nc.dram_tensor('wu3', [64, 128], f32, kind="ExternalInput").ap()
    wt['wv3'] = nc.dram_tensor('wv3', [64, 128], f32, kind="ExternalInput").ap()
    wt['bv3'] = nc.dram_tensor('bv3', [128, 1], f32, kind="ExternalInput").ap()
    w1t = nc.dram_tensor("w1t", [320, 1024], f32, kind="ExternalInput").ap()
    b1 = nc.dram_tensor("b1", [128, 8], f32, kind="ExternalInput").ap()
    w2t = nc.dram_tensor("w2t", [1024, 512], f32, kind="ExternalInput").ap()
    b2 = nc.dram_tensor("b2", [128, 4], f32, kind="ExternalInput").ap()
    w3t = nc.dram_tensor("w3t", [512, 40], f32, kind="ExternalInput").ap()
    b3 = nc.dram_tensor("b3", [40, 1], f32, kind="ExternalInput").ap()

    out_d = nc.dram_tensor("out", [40, BPC], f32, kind="ExternalOutput").ap()
    stage = nc.dram_tensor("idx_stage", [BPC, N, K], u16, kind="Internal").ap()
    pooled_d = nc.dram_tensor("pooled_stage", [BPC, 320], f32, kind="Internal").ap()

    NPAIR = BPC // 2

    with TileContext(nc) as tc:
        with (
            tc.tile_pool(name="const", bufs=1) as cpool,
            tc.tile_pool(name="wpool", bufs=1) as wpool,
            tc.tile_pool(name="feat", bufs=1) as fpool,
            tc.tile_pool(name="work", bufs=2) as wkpool,
            tc.tile_pool(name="pdp", bufs=6) as pdpool,
            tc.tile_pool(name="sel", bufs=6) as selpool,
            tc.tile_pool(name="gath", bufs=2) as gpool,
            tc.tile_pool(name="ps", bufs=2, space="PSUM") as pspool,
            tc.tile_pool(name="psx", bufs=1, space="PSUM") as psxpool,
        ):
            ones_col = cpool.tile([128, 1], f32, tag="onesc")
            nc.vector.memset(ones_col[:, :], 1.0)
            ones_row = cpool.tile([1, N], f32, tag="onesr")
            nc.vector.memset(ones_row[:, :], 1.0)

            # load weights (all at base partition 0 — the PE requires matmul
            # operands to share a base partition, and mixing tile_positions
            # inside one PSUM accumulation group faults on HW)
            wsb = {}
            for l in range(3):
                for key in (f'wu{l}a', f'wu{l}b', f'wv{l}a', f'wv{l}b'):
                    t = wpool.tile([cins[l], 128], f32, tag=key, name=key)
                    nc.sync.dma_start(t[:, :], wt[key])
                    wsb[key] = t
                t = wpool.tile([128, 1], f32, tag=f'bv{l}', name=f'bv{l}')
                nc.sync.dma_start(t[:, :], wt[f'bv{l}'])
                wsb[f'bv{l}'] = t
            for key in ('wu3', 'wv3'):
                t = wpool.tile([64, 128], f32, tag=key, name=key)
                nc.sync.dma_start(t[:, :], wt[key])
                wsb[key] = t
            t = wpool.tile([128, 1], f32, tag='bv3', name='bv3')
            nc.sync.dma_start(t[:, :], wt['bv3'])
            wsb['bv3'] = t

            # Feature state per pair: paired tile F[p] (128, N) holds unit A
            # in partitions [0:64); FB[p] (64, N) is unit B's copy at base 0
            # (extracted by DMA) so every matmul operand starts at partition 0.
            F = [fpool.tile([128, N], f32, tag=f"F{p}", name=f"F{p}", bufs=2)
                 for p in range(NPAIR)]
            FB = [fpool.tile([64, N], f32, tag=f"FB{p}", name=f"FB{p}", bufs=2)
                  for p in range(NPAIR)]
            for p in range(NPAIR):
                nc.sync.dma_start(F[p][0:3, :], xT[2 * p, :, :])
                nc.sync.dma_start(FB[p][0:3, :], xT[2 * p + 1, :, :])

            for l in range(n_layers):
                C = cins[l]
                for p in range(NPAIR):
                    Fp = F[p]
                    FBp = FB[p]
                    funits = (Fp, FBp)  # unit -> feature AP source (base 0)
                    # ---- squared norms (per unit, base partition 0) ----
                    negxx = [None, None]
                    for ui in range(2):
                        fsq = wkpool.tile([64, N], f32, tag=f"fsq{ui}",
                                          name=f"fsq{ui}")
                        nc.scalar.activation(fsq[0:C, :], funits[ui][0:C, :], AF.Square)
                        xxp = psxpool.tile([1, N], f32, tag="xx", name="xxp")
                        for h in range(2):
                            sl = slice(h * 512, (h + 1) * 512)
                            nc.tensor.matmul(xxp[:, sl], ones_col[0:C, :],
                                             fsq[0:C, sl], start=True, stop=True)
                        nxx = wkpool.tile([1, N], f32, tag=f"nxx{ui}", name=f"nxx{ui}")
                        nc.scalar.activation(nxx[:, :], xxp[:, :], AF.Copy, scale=-1.0)
                        negxx[ui] = nxx

                    # ---- u/v feature tables ----
                    if l < 3:
                        # batch-pair packed: psum = [u_A ; u_B] via padded weights
                        upair = wkpool.tile([128, N], f32, tag="upair")
                        vpair = wkpool.tile([128, N], f32, tag="vpair")
                        for h in range(2):
                            sl = slice(h * 512, (h + 1) * 512)
                            up = pspool.tile([128, 512], f32, tag="acc")
                            vp = pspool.tile([128, 512], f32, tag="acc")
                            nc.tensor.matmul(up[:, :], wsb[f'wu{l}a'][:, :], Fp[0:C, sl],
                                             start=True, stop=False)
                            nc.tensor.matmul(up[:, :], wsb[f'wu{l}b'][:, :],
                                             FBp[0:C, sl], start=False, stop=True)
                            nc.tensor.matmul(vp[:, :], wsb[f'wv{l}a'][:, :], Fp[0:C, sl],
                                             start=True, stop=False)
                            nc.tensor.matmul(vp[:, :], wsb[f'wv{l}b'][:, :],
                                             FBp[0:C, sl], start=False, stop=True)
                            nc.scalar.activation(upair[:, sl], up[:, :], AF.Copy)
                            nc.scalar.activation(vpair[:, sl], vp[:, :], AF.Identity,
                                                 bias=wsb[f'bv{l}'][:, :])
                            del up, vp
                    else:
                        # layer 4: Cout=128 -> per-unit full-width tables
                        u4s, v4s = [], []
                        for ui in range(2):
                            u4 = wkpool.tile([128, N], f32, tag="upair", name=f"u4_{ui}")
                            v4 = wkpool.tile([128, N], f32, tag="vpair", name=f"v4_{ui}")
                            for h in range(2):
                                sl = slice(h * 512, (h + 1) * 512)
                                up = pspool.tile([128, 512], f32, tag="acc")
                                vp = pspool.tile([128, 512], f32, tag="acc")
                                nc.tensor.matmul(up[:, :], wsb['wu3'][:, :],
                                                 funits[ui][0:C, sl], start=True, stop=True)
                                nc.tensor.matmul(vp[:, :], wsb['wv3'][:, :],
                                                 funits[ui][0:C, sl], start=True, stop=True)
                                nc.scalar.activation(u4[:, sl], up[:, :], AF.Copy)
                                nc.scalar.activation(v4[:, sl], vp[:, :], AF.Identity,
                                                     bias=wsb['bv3'][:, :])
                                del up, vp
                            u4s.append(u4)
                            v4s.append(v4)

                    for ui in range(2):
                        b = 2 * p + ui
                        FX = funits[ui]
                        # ---- pd + top-20 selection per 128-row chunk ----
                        for ic in range(8):
                            isl = slice(ic * 128, (ic + 1) * 128)
                            pdp = pspool.tile([128, 1024], f32, tag="pd")
                            for h in range(2):
                                sl = slice(h * 512, (h + 1) * 512)
                                nc.tensor.matmul(pdp[:, sl], FX[0:C, isl],
                                                 FX[0:C, sl], start=True, stop=False)
                                nc.tensor.matmul(pdp[:, sl], FX[0:C, isl],
                                                 FX[0:C, sl], start=False, stop=False)
                                nc.tensor.matmul(pdp[:, sl], negxx[ui][:, isl],
                                                 ones_row[:, sl], start=False, stop=False)
                                nc.tensor.matmul(pdp[:, sl], ones_row[:, isl],
                                                 negxx[ui][:, sl], start=False, stop=True)
                            pda = pdpool.tile([128, 1024], f32, tag="pda")
                            nc.scalar.activation(pda[:, :], pdp[:, :], AF.Copy)
                            del pdp

                            v0 = selpool.tile([128, 8], f32, tag="v0")
                            v1 = selpool.tile([128, 8], f32, tag="v1")
                            v2 = selpool.tile([128, 8], f32, tag="v2")
                            i0 = selpool.tile([128, 8], u16, tag="i0")
                            i1 = selpool.tile([128, 8], u16, tag="i1")
                            i2 = selpool.tile([128, 8], u16, tag="i2")
                            nc.vector.max(out=v0[:, :], in_=pda[:, :])
                            nc.vector.max_index(out=i0[:, :], in_max=v0[:, :], in_values=pda[:, :])
                            pdb = pdpool.tile([128, 1024], f32, tag="pdb")
                            nc.vector.match_replace(out=pdb[:, :], in_to_replace=v0[:, :],
                                                    in_values=pda[:, :], imm_value=NEG)
                            nc.vector.max(out=v1[:, :], in_=pdb[:, :])
                            nc.vector.max_index(out=i1[:, :], in_max=v1[:, :], in_values=pdb[:, :])
                            nc.vector.match_replace(out=pda[:, :], in_to_replace=v1[:, :],
                                                    in_values=pdb[:, :], imm_value=NEG)
                            nc.vector.max(out=v2[:, :], in_=pda[:, :])
                            nc.vector.max_index(out=i2[:, :], in_max=v2[:, :], in_values=pda[:, :])
                            # stage the 20 indices with 3 DMAs (SP engine) so
                            # the DVE does no assembly copies
                            nc.sync.dma_start(stage[b, isl, 0:8], i0[:, :])
                            nc.sync.dma_start(stage[b, isl, 8:16], i1[:, :])
                            nc.sync.dma_start(stage[b, isl, 16:20], i2[:, 0:4])

                    # ---- gather + window-max + v + relu ----
                    def window_max(G, out_slice):
                        nc.vector.reduce_max(
                            out=out_slice,
                            in_=G.rearrange("p (i t) -> p i t", t=K),
                            axis=AX.X)

                    if l < 3:
                        wrap = gpool.tile([128, N * K // 16], u16, tag="wrap")
                        for g in range(8):
                            bsrc = 2 * p + (0 if g < 4 else 1)
                            lin = stage[bsrc].rearrange("i t -> (i t)").rearrange(
                                "(c r) -> r c", r=16)
                            nc.sync.dma_start(wrap[g * 16:(g + 1) * 16, :], lin)
                        Mp = wkpool.tile([128, N], f32, tag="Mp")
                        for gc in range(8):
                            G = gpool.tile([128, 2560], f32, tag="G", bufs=3)
                            nc.gpsimd.ap_gather(
                                out_ap=G[:, :], in_ap=upair[:, :],
                                idxs_ap=wrap[:, gc * 160:(gc + 1) * 160].bitcast(i16),
                                channels=128, num_elems=N, d=1, num_idxs=2560)
                            window_max(G, Mp[:, gc * 128:(gc + 1) * 128])
                        nc.vector.tensor_add(Mp[:, :], Mp[:, :], vpair[:, :])
                        Fnext = fpool.tile([128, N], f32, tag=f"F{p}",
                                           name=f"F{p}_{l}", bufs=2)
                        nc.scalar.activation(Fnext[:, :], Mp[:, :], AF.Relu)
                        FBnext = fpool.tile([64, N], f32, tag=f"FB{p}",
                                            name=f"FB{p}_{l}", bufs=2)
                        nc.sync.dma_start(FBnext[:, :], Fnext[64:128, :])
                        # global max-pool for this layer
                        gp = selpool.tile([128, 1], f32, tag="gp")
                        nc.vector.reduce_max(out=gp[:, :], in_=Fnext[:, :], axis=AX.X)
                        nc.sync.dma_start(pooled_d[2 * p, l * 64:(l + 1) * 64], gp[0:64, :])
                        nc.sync.dma_start(pooled_d[2 * p + 1, l * 64:(l + 1) * 64], gp[64:128, :])
                        F[p], FB[p] = Fnext, FBnext
                    else:
                        for ui in range(2):
                            b = 2 * p + ui
                            wrap = gpool.tile([128, N * K // 16], u16, tag="wrap")
                            lin = stage[b].rearrange("i t -> (i t)").rearrange(
                                "(c r) -> r c", r=16)
                            for g in range(8):
                                nc.sync.dma_start(wrap[g * 16:(g + 1) * 16, :], lin)
                            Mp = wkpool.tile([128, N], f32, tag="Mp")
                            for gc in range(8):
                                G = gpool.tile([128, 2560], f32, tag="G", bufs=3)
                                nc.gpsimd.ap_gather(
                                    out_ap=G[:, :], in_ap=u4s[ui][:, :],
                                    idxs_ap=wrap[:, gc * 160:(gc + 1) * 160].bitcast(i16),
                                    channels=128, num_elems=N, d=1, num_idxs=2560)
                                window_max(G, Mp[:, gc * 128:(gc + 1) * 128])
                            nc.vector.tensor_add(Mp[:, :], Mp[:, :], v4s[ui][:, :])
                            x4t = wkpool.tile([128, N], f32, tag="x4t")
                            nc.scalar.activation(x4t[:, :], Mp[:, :], AF.Relu)
                            gp = selpool.tile([128, 1], f32, tag="gp")
                            nc.vector.reduce_max(out=gp[:, :], in_=x4t[:, :], axis=AX.X)
                            nc.sync.dma_start(pooled_d[b, 192:320], gp[:, :])

        # ================= MLP head (own pool scope) =================
        if not with_mlp:
            with tc.tile_pool(name="stub", bufs=1) as spool:
                so = spool.tile([40, BPC], f32, name="so")
                nc.sync.dma_start(so[:, :], pooled_d[:, 0:40].rearrange("b p -> p b"))
                nc.sync.dma_start(out_d, so[:, :])
        elif True:
          with (
            tc.tile_pool(name="mlp", bufs=1) as mpool,
            tc.tile_pool(name="mps", bufs=2, space="PSUM") as mpspool,
          ):
            pooledT = mpool.tile([128, 3, BPC], f32, tag="pooledT")
            for kc in range(3):
                kn = 128 if kc < 2 else 64
                nc.sync.dma_start(pooledT[0:kn, kc, :],
                                  pooled_d[:, kc * 128:kc * 128 + kn].rearrange("b p -> p b"))
            w1sb = mpool.tile([128, 3, 1024], f32, tag="w1sb")
            for kc in range(3):
                kn = 128 if kc < 2 else 64
                nc.sync.dma_start(w1sb[0:kn, kc, :], w1t[kc * 128:kc * 128 + kn, :])
            b1sb = mpool.tile([128, 8], f32, tag="b1sb")
            nc.sync.dma_start(b1sb[:, :], b1)
            h1 = mpool.tile([128, 8, BPC], f32, tag="h1")
            for mc in range(8):
                hp = mpspool.tile([128, BPC], f32, tag="acc")
                for kc in range(3):
                    kn = 128 if kc < 2 else 64
                    nc.tensor.matmul(hp[:, :], w1sb[0:kn, kc, mc * 128:(mc + 1) * 128],
                                     pooledT[0:kn, kc, :], start=(kc == 0), stop=(kc == 2))
                nc.scalar.activation(h1[:, mc, :], hp[:, :], AF.Relu,
                                     bias=b1sb[:, mc:mc + 1])
            w2sb = mpool.tile([128, 8, 512], f32, tag="w2sb")
            for kc in range(8):
                nc.sync.dma_start(w2sb[:, kc, :], w2t[kc * 128:(kc + 1) * 128, :])
            b2sb = mpool.tile([128, 4], f32, tag="b2sb")
            nc.sync.dma_start(b2sb[:, :], b2)
            h2 = mpool.tile([128, 4, BPC], f32, tag="h2")
            for mc in range(4):
                hp = mpspool.tile([128, BPC], f32, tag="acc")
                for kc in range(8):
                    nc.tensor.matmul(hp[:, :], w2sb[:, kc, mc * 128:(mc + 1) * 128],
                                     h1[:, kc, :], start=(kc == 0), stop=(kc == 7))
                nc.scalar.activation(h2[:, mc, :], hp[:, :], AF.Relu,
                                     bias=b2sb[:, mc:mc + 1])
            w3sb = mpool.tile([128, 4, 40], f32, tag="w3sb")
            for kc in range(4):
                nc.sync.dma_start(w3sb[:, kc, :], w3t[kc * 128:(kc + 1) * 128, :])
            b3sb = mpool.tile([40, 1], f32, tag="b3sb")
            nc.sync.dma_start(b3sb[:, :], b3)
            outp = mpspool.tile([40, BPC], f32, tag="acc")
            for kc in range(4):
                nc.tensor.matmul(outp[:, :], w3sb[:, kc, :], h2[:, kc, :],
                                 start=(kc == 0), stop=(kc == 3))
            outsb = mpool.tile([40, BPC], f32, tag="outsb")
            nc.scalar.activation(outsb[:, :], outp[:, :], AF.Identity, bias=b3sb[:, :])
            nc.sync.dma_start(out_d, outsb[:, :])

    nc.compile()
    return nc


# ---------------------------------------------------------------- entry point
def _run(inputs, **spmd_kwargs):
    key = "prog"
    if key not in _CACHE:
        _CACHE[key] = _build_program()
    nc = _CACHE[key]

    inputs = {k: np.asarray(v) for k, v in inputs.items()}
    w = _prep_weights(inputs)
    x = np.asarray(inputs['x'], dtype=np.float32)   # (32, 1024, 3)
    in_maps = []
    for c in range(NCORES):
        xs = x[c * BPC:(c + 1) * BPC]                       # (4, 1024, 3)
        m = {'xT': np.ascontiguousarray(xs.transpose(0, 2, 1)).astype(np.float32)}
        m.update({k: np.ascontiguousarray(v) for k, v in w.items()})
        in_maps.append(m)

    from concourse.bass_utils import run_bass_kernel_spmd
    res = run_bass_kernel_spmd(nc, in_maps, core_ids=list(range(NCORES)), **spmd_kwargs)
    out = np.concatenate([r['out'].T for r in res.results], axis=0)  # (32, 40)
    return out.astype(np.float32), res


def kernel(**inputs):
    return _run(inputs)[0]



# revision 3
# speedup vs baseline: 19.1547x; 19.1547x over previous
"""DGCNN forward on 8 Trainium2 NeuronCores, data-parallel over batch.

Contract: kernel(**inputs) takes the FULL (unsharded) inputs from
reference.setup_inputs() and returns the FULL (32, 40) output.

Algorithm (exact, fp32):
  EdgeConv(x)_i = max_{j in knn20(i)} relu(bn(W @ [x_j - x_i; x_i]))
 decomposes (relu/max commute, bn is affine) into
  u_j = s*(wA @ x_j);  v_i = s*((wB-wA) @ x_i) + b
  out_i = relu( max_{j in knn20(i)} u_j  +  v_i )
 so each layer is: pairwise-distance matmul (PE) -> exact top-20 row
 selection (DVE max8/match_replace/max_index) -> gather u rows by index
 (GPSIMD ap_gather) -> windowed max (DVE reduce) -> +v, relu (ACT).

Dispatch: the wall-clock cost of a call is dominated by host/axon
overhead, not device compute, so
  - all weights are baked into the NEFF as Const tensors (DMA'd to HBM
    once at model load); the only runtime input is xT (48 KB/core);
  - the jitted shard_map executable is memoized across calls (the stock
    run_bass_via_pjrt rebuilds + reloads it every call);
  - the jax persistent compilation cache is enabled so a fresh process
    skips the walrus compile.
"""

import hashlib
import numpy as np

B, N, K = 32, 1024, 20
EPS = 1e-5
NCORES = 8
BPC = B // NCORES          # batches per core
NEG = -1e30

_CACHE = {}


def _setup_jax():
    if '_jax' in _CACHE:
        return
    import jax
    jax.config.update("jax_compilation_cache_dir", "/tmp/bass_jax_cache")
    jax.config.update("jax_persistent_cache_min_compile_time_secs", 0.0)
    jax.config.update("jax_persistent_cache_min_entry_size_bytes", 0)
    _CACHE['_jax'] = True


# ---------------------------------------------------------------- weight prep
def _prep_weights(inp):
    """Fold BN into the edge-conv and MLP weights (numpy, host-side)."""
    w = {}
    couts = [64, 64, 64, 128]
    cins = [3, 64, 64, 64]
    for l in range(4):
        wl = inp[f'w{l+1}']            # (Cout, 2C)
        g = inp[f'g{l+1}']
        b = inp[f'b{l+1}']
        C = cins[l]
        s = g / np.sqrt(1.0 + EPS)
        wA = wl[:, :C]                  # acts on (x_j - x_i)
        wB = wl[:, C:]                  # acts on x_i
        Wu = (s[:, None] * wA).T.astype(np.float32)           # (C, Cout)
        Wv = (s[:, None] * (wB - wA)).T.astype(np.float32)    # (C, Cout)
        cout = couts[l]
        if l < 3:
            # batch-pair packing: [Wu | 0] and [0 | Wu], (C, 128)
            zu = np.zeros((C, 64), np.float32)
            w[f'wu{l}a'] = np.concatenate([Wu, zu], 1)
            w[f'wu{l}b'] = np.concatenate([zu, Wu], 1)
            w[f'wv{l}a'] = np.concatenate([Wv, zu], 1)
            w[f'wv{l}b'] = np.concatenate([zu, Wv], 1)
            w[f'bv{l}'] = np.concatenate([b, b]).reshape(128, 1).astype(np.float32)
        else:
            w[f'wu{l}'] = Wu            # (64, 128)
            w[f'wv{l}'] = Wv
            w[f'bv{l}'] = b.reshape(128, 1).astype(np.float32)
    s5 = inp['g5'] / np.sqrt(1.0 + EPS)
    w['w1t'] = (s5[:, None] * inp['lw1']).T.astype(np.float32)      # (320, 1024)
    w['b1'] = (s5 * inp['lb1'] + inp['b5']).reshape(8, 128).T.astype(np.float32).copy()  # (128, 8)
    s6 = inp['g6'] / np.sqrt(1.0 + EPS)
    w['w2t'] = (s6[:, None] * inp['lw2']).T.astype(np.float32)      # (1024, 512)
    w['b2'] = (s6 * inp['lb2'] + inp['b6']).reshape(4, 128).T.astype(np.float32).copy()  # (128, 4)
    w['w3t'] = inp['lw3'].T.astype(np.float32)                      # (512, 40)
    w['b3'] = inp['lb3'].reshape(40, 1).astype(np.float32)
    return w


# ---------------------------------------------------------------- bass program
def _build_program(w):
    """Build the SPMD program with the weights in `w` baked in as NEFF
    constants. Only xT is a runtime input."""
    import concourse.bass as bass
    import concourse.bacc as bacc
    import concourse.mybir as mybir
    from concourse.tile import TileContext

    f32 = mybir.dt.float32
    u16 = mybir.dt.uint16
    i16 = mybir.dt.int16
    AF = mybir.ActivationFunctionType
    AX = mybir.AxisListType

    nc = bacc.Bacc("TRN2")

    # ---- DRAM tensors ----
    xT = nc.dram_tensor("xT", [BPC, 3, N], f32, kind="ExternalInput").ap()
    cins = [3, 64, 64, 64]
    wt = {k: nc.inline_tensor(np.ascontiguousarray(v), name=f"cw_{k}").ap()
          for k, v in w.items()}

    out_d = nc.dram_tensor("out", [40, BPC], f32, kind="ExternalOutput").ap()
    stage = nc.dram_tensor("idx_stage", [BPC, N, K], u16, kind="Internal").ap()
    pooled_d = nc.dram_tensor("pooled_stage", [BPC, 320], f32, kind="Internal").ap()

    NPAIR = BPC // 2

    with TileContext(nc) as tc:
        with (
            tc.tile_pool(name="const", bufs=1) as cpool,
            tc.tile_pool(name="wpool", bufs=1) as wpool,
            tc.tile_pool(name="feat", bufs=1) as fpool,
            tc.tile_pool(name="work", bufs=2) as wkpool,
            tc.tile_pool(name="pdp", bufs=6) as pdpool,
            tc.tile_pool(name="sel", bufs=6) as selpool,
            tc.tile_pool(name="gath", bufs=2) as gpool,
            tc.tile_pool(name="ps", bufs=2, space="PSUM") as pspool,
            tc.tile_pool(name="psx", bufs=1, space="PSUM") as psxpool,
        ):
            ones_col = cpool.tile([128, 1], f32, tag="onesc")
            nc.vector.memset(ones_col[:, :], 1.0)
            ones_row = cpool.tile([1, N], f32, tag="onesr")
            nc.vector.memset(ones_row[:, :], 1.0)

            # load weights (all at base partition 0 — the PE requires matmul
            # operands to share a base partition, and mixing tile_positions
            # inside one PSUM accumulation group faults on HW)
            wsb = {}
            for l in range(3):
                for key in (f'wu{l}a', f'wu{l}b', f'wv{l}a', f'wv{l}b'):
                    t = wpool.tile([cins[l], 128], f32, tag=key, name=key)
                    nc.sync.dma_start(t[:, :], wt[key])
                    wsb[key] = t
                t = wpool.tile([128, 1], f32, tag=f'bv{l}', name=f'bv{l}')
                nc.sync.dma_start(t[:, :], wt[f'bv{l}'])
                wsb[f'bv{l}'] = t
            for key in ('wu3', 'wv3'):
                t = wpool.tile([64, 128], f32, tag=key, name=key)
                nc.sync.dma_start(t[:, :], wt[key])
                wsb[key] = t
            t = wpool.tile([128, 1], f32, tag='bv3', name='bv3')
            nc.sync.dma_start(t[:, :], wt['bv3'])
            wsb['bv3'] = t

            # Feature state per pair: paired tile F[p] (128, N) holds unit A
            # in partitions [0:64); FB[p] (64, N) is unit B's copy at base 0
            # (extracted by DMA) so every matmul operand starts at partition 0.
            F = [fpool.tile([128, N], f32, tag=f"F{p}", name=f"F{p}", bufs=2)
                 for p in range(NPAIR)]
            FB = [fpool.tile([64, N], f32, tag=f"FB{p}", name=f"FB{p}", bufs=2)
                  for p in range(NPAIR)]
            for p in range(NPAIR):
                nc.sync.dma_start(F[p][0:3, :], xT[2 * p, :, :])
                nc.sync.dma_start(FB[p][0:3, :], xT[2 * p + 1, :, :])

            for l in range(4):
                C = cins[l]
                for p in range(NPAIR):
                    Fp = F[p]
                    FBp = FB[p]
                    funits = (Fp, FBp)  # unit -> feature AP source (base 0)
                    # ---- squared norms (per unit, base partition 0) ----
                    negxx = [None, None]
                    for ui in range(2):
                        fsq = wkpool.tile([64, N], f32, tag=f"fsq{ui}",
                                          name=f"fsq{ui}")
                        nc.scalar.activation(fsq[0:C, :], funits[ui][0:C, :], AF.Square)
                        xxp = psxpool.tile([1, N], f32, tag="xx", name="xxp")
                        for h in range(2):
                            sl = slice(h * 512, (h + 1) * 512)
                            nc.tensor.matmul(xxp[:, sl], ones_col[0:C, :],
                                             fsq[0:C, sl], start=True, stop=True)
                        nxx = wkpool.tile([1, N], f32, tag=f"nxx{ui}", name=f"nxx{ui}")
                        nc.scalar.activation(nxx[:, :], xxp[:, :], AF.Copy, scale=-1.0)
                        negxx[ui] = nxx

                    # ---- u/v feature tables ----
                    if l < 3:
                        # batch-pair packed: psum = [u_A ; u_B] via padded weights
                        upair = wkpool.tile([128, N], f32, tag="upair")
                        vpair = wkpool.tile([128, N], f32, tag="vpair")
                        for h in range(2):
                            sl = slice(h * 512, (h + 1) * 512)
                            up = pspool.tile([128, 512], f32, tag="acc")
                            vp = pspool.tile([128, 512], f32, tag="acc")
                            nc.tensor.matmul(up[:, :], wsb[f'wu{l}a'][:, :], Fp[0:C, sl],
                                             start=True, stop=False)
                            nc.tensor.matmul(up[:, :], wsb[f'wu{l}b'][:, :],
                                             FBp[0:C, sl], start=False, stop=True)
                            nc.tensor.matmul(vp[:, :], wsb[f'wv{l}a'][:, :], Fp[0:C, sl],
                                             start=True, stop=False)
                            nc.tensor.matmul(vp[:, :], wsb[f'wv{l}b'][:, :],
                                             FBp[0:C, sl], start=False, stop=True)
                            nc.scalar.activation(upair[:, sl], up[:, :], AF.Copy)
                            nc.scalar.activation(vpair[:, sl], vp[:, :], AF.Identity,
                                                 bias=wsb[f'bv{l}'][:, :])
                            del up, vp
                    else:
                        # layer 4: Cout=128 -> per-unit full-width tables
                        u4s, v4s = [], []
                        for ui in range(2):
                            u4 = wkpool.tile([128, N], f32, tag="upair", name=f"u4_{ui}")
                            v4 = wkpool.tile([128, N], f32, tag="vpair", name=f"v4_{ui}")
                            for h in range(2):
                                sl = slice(h * 512, (h + 1) * 512)
                                up = pspool.tile([128, 512], f32, tag="acc")
                                vp = pspool.tile([128, 512], f32, tag="acc")
                                nc.tensor.matmul(up[:, :], wsb['wu3'][:, :],
                                                 funits[ui][0:C, sl], start=True, stop=True)
                                nc.tensor.matmul(vp[:, :], wsb['wv3'][:, :],
                                                 funits[ui][0:C, sl], start=True, stop=True)
                                nc.scalar.activation(u4[:, sl], up[:, :], AF.Copy)
                                nc.scalar.activation(v4[:, sl], vp[:, :], AF.Identity,
                                                     bias=wsb['bv3'][:, :])
                                del up, vp
                            u4s.append(u4)
                            v4s.append(v4)

                    for ui in range(2):
                        b = 2 * p + ui
                        FX = funits[ui]
                        # ---- pd + top-20 selection per 128-row chunk ----
                        for ic in range(8):
                            isl = slice(ic * 128, (ic + 1) * 128)
                            pdp = pspool.tile([128, 1024], f32, tag="pd")
                            for h in range(2):
                                sl = slice(h * 512, (h + 1) * 512)
                                nc.tensor.matmul(pdp[:, sl], FX[0:C, isl],
                                                 FX[0:C, sl], start=True, stop=False)
                                nc.tensor.matmul(pdp[:, sl], FX[0:C, isl],
                                                 FX[0:C, sl], start=False, stop=False)
                                nc.tensor.matmul(pdp[:, sl], negxx[ui][:, isl],
                                                 ones_row[:, sl], start=False, stop=False)
                                nc.tensor.matmul(pdp[:, sl], ones_row[:, isl],
                                                 negxx[ui][:, sl], start=False, stop=True)
                            pda = pdpool.tile([128, 1024], f32, tag="pda")
                            nc.scalar.activation(pda[:, :], pdp[:, :], AF.Copy)
                            del pdp

                            v0 = selpool.tile([128, 8], f32, tag="v0")
                            v1 = selpool.tile([128, 8], f32, tag="v1")
                            v2 = selpool.tile([128, 8], f32, tag="v2")
                            i0 = selpool.tile([128, 8], u16, tag="i0")
                            i1 = selpool.tile([128, 8], u16, tag="i1")
                            i2 = selpool.tile([128, 8], u16, tag="i2")
                            nc.vector.max(out=v0[:, :], in_=pda[:, :])
                            nc.vector.max_index(out=i0[:, :], in_max=v0[:, :], in_values=pda[:, :])
                            pdb = pdpool.tile([128, 1024], f32, tag="pdb")
                            nc.vector.match_replace(out=pdb[:, :], in_to_replace=v0[:, :],
                                                    in_values=pda[:, :], imm_value=NEG)
                            nc.vector.max(out=v1[:, :], in_=pdb[:, :])
                            nc.vector.max_index(out=i1[:, :], in_max=v1[:, :], in_values=pdb[:, :])
                            nc.vector.match_replace(out=pda[:, :], in_to_replace=v1[:, :],
                                                    in_values=pdb[:, :], imm_value=NEG)
                            nc.vector.max(out=v2[:, :], in_=pda[:, :])
                            nc.vector.max_index(out=i2[:, :], in_max=v2[:, :], in_values=pda[:, :])
                            # stage the 20 indices with 3 DMAs (SP engine) so
                            # the DVE does no assembly copies
                            nc.sync.dma_start(stage[b, isl, 0:8], i0[:, :])
                            nc.sync.dma_start(stage[b, isl, 8:16], i1[:, :])
                            nc.sync.dma_start(stage[b, isl, 16:20], i2[:, 0:4])

                    # ---- gather + window-max + v + relu ----
                    def window_max(G, out_slice):
                        nc.vector.reduce_max(
                            out=out_slice,
                            in_=G.rearrange("p (i t) -> p i t", t=K),
                            axis=AX.X)

                    if l < 3:
                        wrap = gpool.tile([128, N * K // 16], u16, tag="wrap")
                        for g in range(8):
                            bsrc = 2 * p + (0 if g < 4 else 1)
                            lin = stage[bsrc].rearrange("i t -> (i t)").rearrange(
                                "(c r) -> r c", r=16)
                            nc.sync.dma_start(wrap[g * 16:(g + 1) * 16, :], lin)
                        Mp = wkpool.tile([128, N], f32, tag="Mp")
                        for gc in range(8):
                            G = gpool.tile([128, 2560], f32, tag="G", bufs=3)
                            nc.gpsimd.ap_gather(
                                out_ap=G[:, :], in_ap=upair[:, :],
                                idxs_ap=wrap[:, gc * 160:(gc + 1) * 160].bitcast(i16),
                                channels=128, num_elems=N, d=1, num_idxs=2560)
                            window_max(G, Mp[:, gc * 128:(gc + 1) * 128])
                        nc.vector.tensor_add(Mp[:, :], Mp[:, :], vpair[:, :])
                        Fnext = fpool.tile([128, N], f32, tag=f"F{p}",
                                           name=f"F{p}_{l}", bufs=2)
                        nc.scalar.activation(Fnext[:, :], Mp[:, :], AF.Relu)
                        FBnext = fpool.tile([64, N], f32, tag=f"FB{p}",
                                            name=f"FB{p}_{l}", bufs=2)
                        nc.sync.dma_start(FBnext[:, :], Fnext[64:128, :])
                        # global max-pool for this layer
                        gp = selpool.tile([128, 1], f32, tag="gp")
                        nc.vector.reduce_max(out=gp[:, :], in_=Fnext[:, :], axis=AX.X)
                        nc.sync.dma_start(pooled_d[2 * p, l * 64:(l + 1) * 64], gp[0:64, :])
                        nc.sync.dma_start(pooled_d[2 * p + 1, l * 64:(l + 1) * 64], gp[64:128, :])
                        F[p], FB[p] = Fnext, FBnext
                    else:
                        for ui in range(2):
                            b = 2 * p + ui
                            wrap = gpool.tile([128, N * K // 16], u16, tag="wrap")
                            lin = stage[b].rearrange("i t -> (i t)").rearrange(
                                "(c r) -> r c", r=16)
                            for g in range(8):
                                nc.sync.dma_start(wrap[g * 16:(g + 1) * 16, :], lin)
                            Mp = wkpool.tile([128, N], f32, tag="Mp")
                            for gc in range(8):
                                G = gpool.tile([128, 2560], f32, tag="G", bufs=3)
                                nc.gpsimd.ap_gather(
                                    out_ap=G[:, :], in_ap=u4s[ui][:, :],
                                    idxs_ap=wrap[:, gc * 160:(gc + 1) * 160].bitcast(i16),
                                    channels=128, num_elems=N, d=1, num_idxs=2560)
                                window_max(G, Mp[:, gc * 128:(gc + 1) * 128])
                            nc.vector.tensor_add(Mp[:, :], Mp[:, :], v4s[ui][:, :])
                            x4t = wkpool.tile([128, N], f32, tag="x4t")
                            nc.scalar.activation(x4t[:, :], Mp[:, :], AF.Relu)
                            gp = selpool.tile([128, 1], f32, tag="gp")
                            nc.vector.reduce_max(out=gp[:, :], in_=x4t[:, :], axis=AX.X)
                            nc.sync.dma_start(pooled_d[b, 192:320], gp[:, :])

        # ================= MLP head (own pool scope) =================
        with (
            tc.tile_pool(name="mlp", bufs=1) as mpool,
            tc.tile_pool(name="mps", bufs=2, space="PSUM") as mpspool,
        ):
            pooledT = mpool.tile([128, 3, BPC], f32, tag="pooledT")
            for kc in range(3):
                kn = 128 if kc < 2 else 64
                nc.sync.dma_start(pooledT[0:kn, kc, :],
                                  pooled_d[:, kc * 128:kc * 128 + kn].rearrange("b p -> p b"))
            w1sb = mpool.tile([128, 3, 1024], f32, tag="w1sb")
            for kc in range(3):
                kn = 128 if kc < 2 else 64
                nc.sync.dma_start(w1sb[0:kn, kc, :], wt['w1t'][kc * 128:kc * 128 + kn, :])
            b1sb = mpool.tile([128, 8], f32, tag="b1sb")
            nc.sync.dma_start(b1sb[:, :], wt['b1'])
            h1 = mpool.tile([128, 8, BPC], f32, tag="h1")
            for mc in range(8):
                hp = mpspool.tile([128, BPC], f32, tag="acc")
                for kc in range(3):
                    kn = 128 if kc < 2 else 64
                    nc.tensor.matmul(hp[:, :], w1sb[0:kn, kc, mc * 128:(mc + 1) * 128],
                                     pooledT[0:kn, kc, :], start=(kc == 0), stop=(kc == 2))
                nc.scalar.activation(h1[:, mc, :], hp[:, :], AF.Relu,
                                     bias=b1sb[:, mc:mc + 1])
            w2sb = mpool.tile([128, 8, 512], f32, tag="w2sb")
            for kc in range(8):
                nc.sync.dma_start(w2sb[:, kc, :], wt['w2t'][kc * 128:(kc + 1) * 128, :])
            b2sb = mpool.tile([128, 4], f32, tag="b2sb")
            nc.sync.dma_start(b2sb[:, :], wt['b2'])
            h2 = mpool.tile([128, 4, BPC], f32, tag="h2")
            for mc in range(4):
                hp = mpspool.tile([128, BPC], f32, tag="acc")
                for kc in range(8):
                    nc.tensor.matmul(hp[:, :], w2sb[:, kc, mc * 128:(mc + 1) * 128],
                                     h1[:, kc, :], start=(kc == 0), stop=(kc == 7))
                nc.scalar.activation(h2[:, mc, :], hp[:, :], AF.Relu,
                                     bias=b2sb[:, mc:mc + 1])
            w3sb = mpool.tile([128, 4, 40], f32, tag="w3sb")
            for kc in range(4):
                nc.sync.dma_start(w3sb[:, kc, :], wt['w3t'][kc * 128:(kc + 1) * 128, :])
            b3sb = mpool.tile([40, 1], f32, tag="b3sb")
            nc.sync.dma_start(b3sb[:, :], wt['b3'])
            outp = mpspool.tile([40, BPC], f32, tag="acc")
            for kc in range(4):
                nc.tensor.matmul(outp[:, :], w3sb[:, kc, :], h2[:, kc, :],
                                 start=(kc == 0), stop=(kc == 3))
            outsb = mpool.tile([40, BPC], f32, tag="outsb")
            nc.scalar.activation(outsb[:, :], outp[:, :], AF.Identity, bias=b3sb[:, :])
            nc.sync.dma_start(out_d, outsb[:, :])

    nc.compile()
    # lowering calls nc.to_json_bytes() on every jit retrace; it is pure
    # for a finished program, so memoize it
    jb = nc.to_json_bytes()
    nc.to_json_bytes = lambda: jb
    return nc


# ------------------------------------------------- memoized pjrt dispatch
def _fast_run_spmd(nc, in_maps, core_ids):
    """run_bass_kernel_spmd with the jitted shard_map executable memoized on
    the Bass object (the stock axon path rebuilds jit + reloads the NEFF on
    every call). Temporarily installs a caching run_bass_via_pjrt and goes
    through run_bass_kernel_spmd per the harness contract."""
    import jax
    import concourse.bass2jax as b2j
    import concourse.mybir as mybir
    from concourse.bass_utils import run_bass_kernel_spmd
    from jax.experimental.shard_map import shard_map
    from jax.sharding import Mesh, PartitionSpec

    n_cores = len(core_ids)
    ent = getattr(nc, "_fast_spmd_ent", None)
    if ent is None:
        b2j.install_neuronx_cc_hook()
        assert nc.dbg_addr is None
        partition_name = (nc.partition_id_tensor.name
                          if nc.partition_id_tensor else None)
        in_names, out_names, out_avals, zero_shapes = [], [], [], []
        for alloc in nc.m.functions[0].allocations:
            if not isinstance(alloc, mybir.MemoryLocationSet):
                continue
            name = alloc.memorylocations[0].name
            if alloc.kind == "ExternalInput":
                if name != partition_name:
                    in_names.append(name)
            elif alloc.kind == "ExternalOutput":
                out_names.append(name)
                shape = tuple(alloc.tensor_shape)
                dtype = mybir.dt.np(alloc.dtype)
                out_avals.append(jax.core.ShapedArray(shape, dtype))
                zero_shapes.append((shape, dtype))
        n_params = len(in_names)
        all_names = tuple(in_names + out_names +
                          ([partition_name] if partition_name else []))
        donate = tuple(range(n_params, n_params + len(out_names)))

        def _body(*args):
            operands = list(args)
            if partition_name is not None:
                operands.append(b2j.partition_id_tensor())
            outs = b2j._bass_exec_p.bind(
                *operands,
                out_avals=tuple(out_avals),
                in_names=all_names,
                out_names=tuple(out_names),
                lowering_input_output_aliases=(),
                sim_require_finite=True,
                sim_require_nnan=True,
                nc=nc,
            )
            return tuple(outs)

        devices = jax.devices()[:n_cores]
        assert len(devices) == n_cores
        mesh = Mesh(np.asarray(devices), ("core",))
        in_specs = (PartitionSpec("core"),) * (n_params + len(out_names))
        out_specs = (PartitionSpec("core"),) * len(out_names)
        sharded = jax.jit(
            shard_map(_body, mesh=mesh, in_specs=in_specs,
                      out_specs=out_specs, check_rep=False),
            donate_argnums=donate, keep_unused=True)
        ent = (in_names[:n_params], out_names, out_avals, zero_shapes, sharded)
        nc._fast_spmd_ent = ent

    in_names, out_names, out_avals, zero_shapes, sharded = ent

    def _cached_run(nc, in_maps, n_cores):
        concat_in = [
            np.concatenate([np.asarray(m[name]) for m in in_maps], axis=0)
            for name in in_names
        ]
        concat_zeros = [np.zeros((n_cores * s[0], *s[1:]), d)
                        for (s, d) in zero_shapes]
        out_arrs = sharded(*concat_in, *concat_zeros)
        return [
            {name: np.asarray(out_arrs[i]).reshape(n_cores, *out_avals[i].shape)[c]
             for i, name in enumerate(out_names)}
            for c in range(n_cores)
        ]

    orig = b2j.run_bass_via_pjrt
    b2j.run_bass_via_pjrt = _cached_run
    try:
        return run_bass_kernel_spmd(nc, in_maps, core_ids=core_ids)
    finally:
        b2j.run_bass_via_pjrt = orig


# ---------------------------------------------------------------- entry point
def _run(inputs, **spmd_kwargs):
    _setup_jax()
    inputs = {k: np.asarray(v) for k, v in inputs.items()}
    w = _prep_weights(inputs)
    wkey = hashlib.sha256(
        b"".join(np.ascontiguousarray(w[k]).tobytes() for k in sorted(w))
    ).hexdigest()
    if wkey not in _CACHE:
        _CACHE[wkey] = _build_program(w)
    nc = _CACHE[wkey]

    x = np.asarray(inputs['x'], dtype=np.float32)   # (32, 1024, 3)
    in_maps = []
    for c in range(NCORES):
        xs = x[c * BPC:(c + 1) * BPC]                       # (4, 1024, 3)
        in_maps.append(
            {'xT': np.ascontiguousarray(xs.transpose(0, 2, 1)).astype(np.float32)})

    if spmd_kwargs:
        from concourse.bass_utils import run_bass_kernel_spmd
        res = run_bass_kernel_spmd(nc, in_maps, core_ids=list(range(NCORES)),
                                   **spmd_kwargs)
    else:
        res = _fast_run_spmd(nc, in_maps, core_ids=list(range(NCORES)))
    out = np.concatenate([r['out'].T for r in res.results], axis=0)  # (32, 40)
    return out.astype(np.float32), res


def kernel(**inputs):
    return _run(inputs)[0]


# revision 5
# speedup vs baseline: 20.4644x; 1.0684x over previous
"""DGCNN forward on 8 Trainium2 NeuronCores, data-parallel over batch.

Contract: kernel(**inputs) takes the FULL (unsharded) inputs from
reference.setup_inputs() and returns the FULL (32, 40) output.

Algorithm (exact, fp32):
  EdgeConv(x)_i = max_{j in knn20(i)} relu(bn(W @ [x_j - x_i; x_i]))
 decomposes (relu/max commute, bn is affine) into
  u_j = s*(wA @ x_j);  v_i = s*((wB-wA) @ x_i) + b
  out_i = relu( max_{j in knn20(i)} u_j  +  v_i )
 so each layer is: pairwise-distance matmul (PE) -> exact top-20 row
 selection (DVE max8/match_replace/max_index) -> gather u rows by index
 (GPSIMD ap_gather) -> windowed max (DVE reduce) -> +v, relu (ACT).

Dispatch: the wall-clock cost of a call is dominated by host/axon
overhead, not device compute, so
  - all weights are baked into the NEFF as Const tensors (DMA'd to HBM
    once at model load); the only runtime input is xT (48 KB/core);
  - the jitted shard_map executable is memoized across calls (the stock
    run_bass_via_pjrt rebuilds + reloads it every call);
  - the jax persistent compilation cache is enabled so a fresh process
    skips the walrus compile.
"""

import hashlib
import numpy as np

B, N, K = 32, 1024, 20
EPS = 1e-5
NCORES = 8
BPC = B // NCORES          # batches per core
NEG = -1e30

_CACHE = {}


def _setup_jax():
    if '_jax' in _CACHE:
        return
    import jax
    jax.config.update("jax_compilation_cache_dir", "/tmp/bass_jax_cache")
    jax.config.update("jax_persistent_cache_min_compile_time_secs", 0.0)
    jax.config.update("jax_persistent_cache_min_entry_size_bytes", 0)
    _CACHE['_jax'] = True


# ---------------------------------------------------------------- weight prep
def _prep_weights(inp):
    """Fold BN into the edge-conv and MLP weights (numpy, host-side)."""
    w = {}
    couts = [64, 64, 64, 128]
    cins = [3, 64, 64, 64]
    for l in range(4):
        wl = inp[f'w{l+1}']            # (Cout, 2C)
        g = inp[f'g{l+1}']
        b = inp[f'b{l+1}']
        C = cins[l]
        s = g / np.sqrt(1.0 + EPS)
        wA = wl[:, :C]                  # acts on (x_j - x_i)
        wB = wl[:, C:]                  # acts on x_i
        Wu = (s[:, None] * wA).T.astype(np.float32)           # (C, Cout)
        Wv = (s[:, None] * (wB - wA)).T.astype(np.float32)    # (C, Cout)
        cout = couts[l]
        if l < 3:
            # batch-pair packing: [Wu | 0] and [0 | Wu], (C, 128)
            zu = np.zeros((C, 64), np.float32)
            w[f'wu{l}a'] = np.concatenate([Wu, zu], 1)
            w[f'wu{l}b'] = np.concatenate([zu, Wu], 1)
            w[f'wv{l}a'] = np.concatenate([Wv, zu], 1)
            w[f'wv{l}b'] = np.concatenate([zu, Wv], 1)
            w[f'bv{l}'] = np.concatenate([b, b]).reshape(128, 1).astype(np.float32)
        else:
            w[f'wu{l}'] = Wu            # (64, 128)
            w[f'wv{l}'] = Wv
            w[f'bv{l}'] = b.reshape(128, 1).astype(np.float32)
    s5 = inp['g5'] / np.sqrt(1.0 + EPS)
    w['w1t'] = (s5[:, None] * inp['lw1']).T.astype(np.float32)      # (320, 1024)
    w['b1'] = (s5 * inp['lb1'] + inp['b5']).reshape(8, 128).T.astype(np.float32).copy()  # (128, 8)
    s6 = inp['g6'] / np.sqrt(1.0 + EPS)
    w['w2t'] = (s6[:, None] * inp['lw2']).T.astype(np.float32)      # (1024, 512)
    w['b2'] = (s6 * inp['lb2'] + inp['b6']).reshape(4, 128).T.astype(np.float32).copy()  # (128, 4)
    w['w3t'] = inp['lw3'].T.astype(np.float32)                      # (512, 40)
    w['b3'] = inp['lb3'].reshape(40, 1).astype(np.float32)
    return w


# ---------------------------------------------------------------- bass program
def _build_program(w):
    """Build the SPMD program with the weights in `w` baked in as NEFF
    constants. Only xT is a runtime input."""
    import concourse.bass as bass
    import concourse.bacc as bacc
    import concourse.mybir as mybir
    from concourse.tile import TileContext

    f32 = mybir.dt.float32
    u16 = mybir.dt.uint16
    i16 = mybir.dt.int16
    AF = mybir.ActivationFunctionType
    AX = mybir.AxisListType

    nc = bacc.Bacc("TRN2")

    # ---- DRAM tensors ----
    xT = nc.dram_tensor("xT", [BPC, 3, N], f32, kind="ExternalInput").ap()
    cins = [3, 64, 64, 64]
    wt = {k: nc.inline_tensor(np.ascontiguousarray(v), name=f"cw_{k}").ap()
          for k, v in w.items()}

    out_d = nc.dram_tensor("out", [40, BPC], f32, kind="ExternalOutput").ap()
    stage = nc.dram_tensor("idx_stage", [BPC, N, K], u16, kind="Internal").ap()
    pooled_d = nc.dram_tensor("pooled_stage", [BPC, 320], f32, kind="Internal").ap()

    NPAIR = BPC // 2

    with TileContext(nc) as tc:
        with (
            tc.tile_pool(name="const", bufs=1) as cpool,
            tc.tile_pool(name="wpool", bufs=1) as wpool,
            tc.tile_pool(name="feat", bufs=1) as fpool,
            tc.tile_pool(name="work", bufs=2) as wkpool,
            tc.tile_pool(name="pdp", bufs=6) as pdpool,
            tc.tile_pool(name="sel", bufs=6) as selpool,
            tc.tile_pool(name="gath", bufs=2) as gpool,
            tc.tile_pool(name="ps", bufs=2, space="PSUM") as pspool,
            tc.tile_pool(name="psx", bufs=1, space="PSUM") as psxpool,
        ):
            ones_col = cpool.tile([128, 1], f32, tag="onesc")
            nc.vector.memset(ones_col[:, :], 1.0)
            ones_row = cpool.tile([1, N], f32, tag="onesr")
            nc.vector.memset(ones_row[:, :], 1.0)

            # load weights (all at base partition 0 — the PE requires matmul
            # operands to share a base partition, and mixing tile_positions
            # inside one PSUM accumulation group faults on HW)
            wsb = {}
            for l in range(3):
                for key in (f'wu{l}a', f'wu{l}b', f'wv{l}a', f'wv{l}b'):
                    t = wpool.tile([cins[l], 128], f32, tag=key, name=key)
                    nc.sync.dma_start(t[:, :], wt[key])
                    wsb[key] = t
                t = wpool.tile([128, 1], f32, tag=f'bv{l}', name=f'bv{l}')
                nc.sync.dma_start(t[:, :], wt[f'bv{l}'])
                wsb[f'bv{l}'] = t
            for key in ('wu3', 'wv3'):
                t = wpool.tile([64, 128], f32, tag=key, name=key)
                nc.sync.dma_start(t[:, :], wt[key])
                wsb[key] = t
            t = wpool.tile([128, 1], f32, tag='bv3', name='bv3')
            nc.sync.dma_start(t[:, :], wt['bv3'])
            wsb['bv3'] = t

            # Feature state per pair: paired tile F[p] (128, N) holds unit A
            # in partitions [0:64); FB[p] (64, N) is unit B's copy at base 0
            # (extracted by DMA) so every matmul operand starts at partition 0.
            F = [fpool.tile([128, N], f32, tag=f"F{p}", name=f"F{p}", bufs=2)
                 for p in range(NPAIR)]
            FB = [fpool.tile([64, N], f32, tag=f"FB{p}", name=f"FB{p}", bufs=2)
                  for p in range(NPAIR)]
            for p in range(NPAIR):
                nc.sync.dma_start(F[p][0:3, :], xT[2 * p, :, :])
                nc.sync.dma_start(FB[p][0:3, :], xT[2 * p + 1, :, :])

            for l in range(4):
                C = cins[l]
                for p in range(NPAIR):
                    Fp = F[p]
                    FBp = FB[p]
                    funits = (Fp, FBp)  # unit -> feature AP source (base 0)
                    # ---- squared norms (per unit, base partition 0) ----
                    negxx = [None, None]
                    for ui in range(2):
                        fsq = wkpool.tile([64, N], f32, tag=f"fsq{ui}",
                                          name=f"fsq{ui}")
                        nc.scalar.activation(fsq[0:C, :], funits[ui][0:C, :], AF.Square)
                        xxp = psxpool.tile([1, N], f32, tag="xx", name="xxp")
                        for h in range(2):
                            sl = slice(h * 512, (h + 1) * 512)
                            nc.tensor.matmul(xxp[:, sl], ones_col[0:C, :],
                                             fsq[0:C, sl], start=True, stop=True)
                        nxx = wkpool.tile([1, N], f32, tag=f"nxx{ui}", name=f"nxx{ui}")
                        nc.scalar.activation(nxx[:, :], xxp[:, :], AF.Copy, scale=-1.0)
                        negxx[ui] = nxx

                    # ---- u/v feature tables ----
                    if l < 3:
                        # batch-pair packed: psum = [u_A ; u_B] via padded weights
                        upair = wkpool.tile([128, N], f32, tag="upair")
                        vpair = wkpool.tile([128, N], f32, tag="vpair")
                        for h in range(2):
                            sl = slice(h * 512, (h + 1) * 512)
                            up = pspool.tile([128, 512], f32, tag="acc")
                            vp = pspool.tile([128, 512], f32, tag="acc")
                            nc.tensor.matmul(up[:, :], wsb[f'wu{l}a'][:, :], Fp[0:C, sl],
                                             start=True, stop=False)
                            nc.tensor.matmul(up[:, :], wsb[f'wu{l}b'][:, :],
                                             FBp[0:C, sl], start=False, stop=True)
                            nc.tensor.matmul(vp[:, :], wsb[f'wv{l}a'][:, :], Fp[0:C, sl],
                                             start=True, stop=False)
                            nc.tensor.matmul(vp[:, :], wsb[f'wv{l}b'][:, :],
                                             FBp[0:C, sl], start=False, stop=True)
                            nc.scalar.activation(upair[:, sl], up[:, :], AF.Copy)
                            nc.scalar.activation(vpair[:, sl], vp[:, :], AF.Identity,
                                                 bias=wsb[f'bv{l}'][:, :])
                            del up, vp
                    else:
                        # layer 4: Cout=128 -> per-unit full-width tables
                        u4s, v4s = [], []
                        for ui in range(2):
                            u4 = wkpool.tile([128, N], f32, tag="upair", name=f"u4_{ui}")
                            v4 = wkpool.tile([128, N], f32, tag="vpair", name=f"v4_{ui}")
                            for h in range(2):
                                sl = slice(h * 512, (h + 1) * 512)
                                up = pspool.tile([128, 512], f32, tag="acc")
                                vp = pspool.tile([128, 512], f32, tag="acc")
                                nc.tensor.matmul(up[:, :], wsb['wu3'][:, :],
                                                 funits[ui][0:C, sl], start=True, stop=True)
                                nc.tensor.matmul(vp[:, :], wsb['wv3'][:, :],
                                                 funits[ui][0:C, sl], start=True, stop=True)
                                nc.scalar.activation(u4[:, sl], up[:, :], AF.Copy)
                                nc.scalar.activation(v4[:, sl], vp[:, :], AF.Identity,
                                                     bias=wsb['bv3'][:, :])
                                del up, vp
                            u4s.append(u4)
                            v4s.append(v4)

                    for ui in range(2):
                        b = 2 * p + ui
                        FX = funits[ui]
                        # ---- pd + top-20 selection per 128-row chunk ----
                        for ic in range(8):
                            isl = slice(ic * 128, (ic + 1) * 128)
                            pdp = pspool.tile([128, 1024], f32, tag="pd")
                            for h in range(2):
                                sl = slice(h * 512, (h + 1) * 512)
                                nc.tensor.matmul(pdp[:, sl], FX[0:C, isl],
                                                 FX[0:C, sl], start=True, stop=False)
                                nc.tensor.matmul(pdp[:, sl], FX[0:C, isl],
                                                 FX[0:C, sl], start=False, stop=False)
                                nc.tensor.matmul(pdp[:, sl], negxx[ui][:, isl],
                                                 ones_row[:, sl], start=False, stop=False)
                                nc.tensor.matmul(pdp[:, sl], ones_row[:, isl],
                                                 negxx[ui][:, sl], start=False, stop=True)
                            pda = pdpool.tile([128, 1024], f32, tag="pda")
                            nc.scalar.activation(pda[:, :], pdp[:, :], AF.Copy)
                            del pdp

                            v0 = selpool.tile([128, 8], f32, tag="v0")
                            v1 = selpool.tile([128, 8], f32, tag="v1")
                            v2 = selpool.tile([128, 8], f32, tag="v2")
                            i0 = selpool.tile([128, 8], u16, tag="i0")
                            i1 = selpool.tile([128, 8], u16, tag="i1")
                            i2 = selpool.tile([128, 8], u16, tag="i2")
                            nc.vector.max(out=v0[:, :], in_=pda[:, :])
                            nc.vector.max_index(out=i0[:, :], in_max=v0[:, :], in_values=pda[:, :])
                            pdb = pdpool.tile([128, 1024], f32, tag="pdb")
                            nc.vector.match_replace(out=pdb[:, :], in_to_replace=v0[:, :],
                                                    in_values=pda[:, :], imm_value=NEG)
                            nc.vector.max(out=v1[:, :], in_=pdb[:, :])
                            nc.vector.max_index(out=i1[:, :], in_max=v1[:, :], in_values=pdb[:, :])
                            nc.vector.match_replace(out=pda[:, :], in_to_replace=v1[:, :],
                                                    in_values=pdb[:, :], imm_value=NEG)
                            nc.vector.max(out=v2[:, :], in_=pda[:, :])
                            nc.vector.max_index(out=i2[:, :], in_max=v2[:, :], in_values=pda[:, :])
                            # stage the 20 indices with 3 DMAs (SP engine) so
                            # the DVE does no assembly copies
                            nc.sync.dma_start(stage[b, isl, 0:8], i0[:, :])
                            nc.sync.dma_start(stage[b, isl, 8:16], i1[:, :])
                            nc.sync.dma_start(stage[b, isl, 16:20], i2[:, 0:4])

                    # ---- gather + window-max + v + relu ----
                    def window_max(G, out_slice):
                        nc.vector.reduce_max(
                            out=out_slice,
                            in_=G.rearrange("p (i t) -> p i t", t=K),
                            axis=AX.X)

                    if l < 3:
                        wrap = gpool.tile([128, N * K // 16], u16, tag="wrap")
                        for g in range(8):
                            bsrc = 2 * p + (0 if g < 4 else 1)
                            lin = stage[bsrc].rearrange("i t -> (i t)").rearrange(
                                "(c r) -> r c", r=16)
                            nc.sync.dma_start(wrap[g * 16:(g + 1) * 16, :], lin)
                        Mp = wkpool.tile([128, N], f32, tag="Mp")
                        for gc in range(8):
                            G = gpool.tile([128, 2560], f32, tag="G", bufs=3)
                            nc.gpsimd.ap_gather(
                                out_ap=G[:, :], in_ap=upair[:, :],
                                idxs_ap=wrap[:, gc * 160:(gc + 1) * 160].bitcast(i16),
                                channels=128, num_elems=N, d=1, num_idxs=2560)
                            window_max(G, Mp[:, gc * 128:(gc + 1) * 128])
                        nc.vector.tensor_add(Mp[:, :], Mp[:, :], vpair[:, :])
                        Fnext = fpool.tile([128, N], f32, tag=f"F{p}",
                                           name=f"F{p}_{l}", bufs=2)
                        nc.scalar.activation(Fnext[:, :], Mp[:, :], AF.Relu)
                        FBnext = fpool.tile([64, N], f32, tag=f"FB{p}",
                                            name=f"FB{p}_{l}", bufs=2)
                        nc.sync.dma_start(FBnext[:, :], Fnext[64:128, :])
                        # global max-pool for this layer
                        gp = selpool.tile([128, 1], f32, tag="gp")
                        nc.vector.reduce_max(out=gp[:, :], in_=Fnext[:, :], axis=AX.X)
                        nc.sync.dma_start(pooled_d[2 * p, l * 64:(l + 1) * 64], gp[0:64, :])
                        nc.sync.dma_start(pooled_d[2 * p + 1, l * 64:(l + 1) * 64], gp[64:128, :])
                        F[p], FB[p] = Fnext, FBnext
                    else:
                        for ui in range(2):
                            b = 2 * p + ui
                            wrap = gpool.tile([128, N * K // 16], u16, tag="wrap")
                            lin = stage[b].rearrange("i t -> (i t)").rearrange(
                                "(c r) -> r c", r=16)
                            for g in range(8):
                                nc.sync.dma_start(wrap[g * 16:(g + 1) * 16, :], lin)
                            Mp = wkpool.tile([128, N], f32, tag="Mp")
                            for gc in range(8):
                                G = gpool.tile([128, 2560], f32, tag="G", bufs=3)
                                nc.gpsimd.ap_gather(
                                    out_ap=G[:, :], in_ap=u4s[ui][:, :],
                                    idxs_ap=wrap[:, gc * 160:(gc + 1) * 160].bitcast(i16),
                                    channels=128, num_elems=N, d=1, num_idxs=2560)
                                window_max(G, Mp[:, gc * 128:(gc + 1) * 128])
                            nc.vector.tensor_add(Mp[:, :], Mp[:, :], v4s[ui][:, :])
                            x4t = wkpool.tile([128, N], f32, tag="x4t")
                            nc.scalar.activation(x4t[:, :], Mp[:, :], AF.Relu)
                            gp = selpool.tile([128, 1], f32, tag="gp")
                            nc.vector.reduce_max(out=gp[:, :], in_=x4t[:, :], axis=AX.X)
                            nc.sync.dma_start(pooled_d[b, 192:320], gp[:, :])

        # ================= MLP head (own pool scope) =================
        with (
            tc.tile_pool(name="mlp", bufs=1) as mpool,
            tc.tile_pool(name="mps", bufs=2, space="PSUM") as mpspool,
        ):
            pooledT = mpool.tile([128, 3, BPC], f32, tag="pooledT")
            for kc in range(3):
                kn = 128 if kc < 2 else 64
                nc.sync.dma_start(pooledT[0:kn, kc, :],
                                  pooled_d[:, kc * 128:kc * 128 + kn].rearrange("b p -> p b"))
            w1sb = mpool.tile([128, 3, 1024], f32, tag="w1sb")
            for kc in range(3):
                kn = 128 if kc < 2 else 64
                nc.sync.dma_start(w1sb[0:kn, kc, :], wt['w1t'][kc * 128:kc * 128 + kn, :])
            b1sb = mpool.tile([128, 8], f32, tag="b1sb")
            nc.sync.dma_start(b1sb[:, :], wt['b1'])
            h1 = mpool.tile([128, 8, BPC], f32, tag="h1")
            for mc in range(8):
                hp = mpspool.tile([128, BPC], f32, tag="acc")
                for kc in range(3):
                    kn = 128 if kc < 2 else 64
                    nc.tensor.matmul(hp[:, :], w1sb[0:kn, kc, mc * 128:(mc + 1) * 128],
                                     pooledT[0:kn, kc, :], start=(kc == 0), stop=(kc == 2))
                nc.scalar.activation(h1[:, mc, :], hp[:, :], AF.Relu,
                                     bias=b1sb[:, mc:mc + 1])
            w2sb = mpool.tile([128, 8, 512], f32, tag="w2sb")
            for kc in range(8):
                nc.sync.dma_start(w2sb[:, kc, :], wt['w2t'][kc * 128:(kc + 1) * 128, :])
            b2sb = mpool.tile([128, 4], f32, tag="b2sb")
            nc.sync.dma_start(b2sb[:, :], wt['b2'])
            h2 = mpool.tile([128, 4, BPC], f32, tag="h2")
            for mc in range(4):
                hp = mpspool.tile([128, BPC], f32, tag="acc")
                for kc in range(8):
                    nc.tensor.matmul(hp[:, :], w2sb[:, kc, mc * 128:(mc + 1) * 128],
                                     h1[:, kc, :], start=(kc == 0), stop=(kc == 7))
                nc.scalar.activation(h2[:, mc, :], hp[:, :], AF.Relu,
                                     bias=b2sb[:, mc:mc + 1])
            w3sb = mpool.tile([128, 4, 40], f32, tag="w3sb")
            for kc in range(4):
                nc.sync.dma_start(w3sb[:, kc, :], wt['w3t'][kc * 128:(kc + 1) * 128, :])
            b3sb = mpool.tile([40, 1], f32, tag="b3sb")
            nc.sync.dma_start(b3sb[:, :], wt['b3'])
            outp = mpspool.tile([40, BPC], f32, tag="acc")
            for kc in range(4):
                nc.tensor.matmul(outp[:, :], w3sb[:, kc, :], h2[:, kc, :],
                                 start=(kc == 0), stop=(kc == 3))
            outsb = mpool.tile([40, BPC], f32, tag="outsb")
            nc.scalar.activation(outsb[:, :], outp[:, :], AF.Identity, bias=b3sb[:, :])
            nc.sync.dma_start(out_d, outsb[:, :])

    nc.compile()
    # lowering calls nc.to_json_bytes() on every jit retrace; it is pure
    # for a finished program, so memoize it
    jb = nc.to_json_bytes()
    nc.to_json_bytes = lambda: jb
    return nc


# ------------------------------------------------- memoized pjrt dispatch
def _fast_run_spmd(nc, in_maps, core_ids):
    """run_bass_kernel_spmd with the jitted shard_map executable memoized on
    the Bass object (the stock axon path rebuilds jit + reloads the NEFF on
    every call). Temporarily installs a caching run_bass_via_pjrt and goes
    through run_bass_kernel_spmd per the harness contract."""
    import jax
    import concourse.bass2jax as b2j
    import concourse.mybir as mybir
    from concourse.bass_utils import run_bass_kernel_spmd
    from jax.experimental.shard_map import shard_map
    from jax.sharding import Mesh, PartitionSpec

    n_cores = len(core_ids)
    ent = getattr(nc, "_fast_spmd_ent", None)
    if ent is None:
        b2j.install_neuronx_cc_hook()
        assert nc.dbg_addr is None
        partition_name = (nc.partition_id_tensor.name
                          if nc.partition_id_tensor else None)
        in_names, out_names, out_avals, zero_shapes = [], [], [], []
        for alloc in nc.m.functions[0].allocations:
            if not isinstance(alloc, mybir.MemoryLocationSet):
                continue
            name = alloc.memorylocations[0].name
            if alloc.kind == "ExternalInput":
                if name != partition_name:
                    in_names.append(name)
            elif alloc.kind == "ExternalOutput":
                out_names.append(name)
                shape = tuple(alloc.tensor_shape)
                dtype = mybir.dt.np(alloc.dtype)
                out_avals.append(jax.core.ShapedArray(shape, dtype))
                zero_shapes.append((shape, dtype))
        n_params = len(in_names)
        all_names = tuple(in_names + out_names +
                          ([partition_name] if partition_name else []))
        donate = tuple(range(n_params, n_params + len(out_names)))

        def _body(*args):
            operands = list(args)
            if partition_name is not None:
                operands.append(b2j.partition_id_tensor())
            outs = b2j._bass_exec_p.bind(
                *operands,
                out_avals=tuple(out_avals),
                in_names=all_names,
                out_names=tuple(out_names),
                lowering_input_output_aliases=(),
                sim_require_finite=True,
                sim_require_nnan=True,
                nc=nc,
            )
            return tuple(outs)

        devices = jax.devices()[:n_cores]
        assert len(devices) == n_cores
        mesh = Mesh(np.asarray(devices), ("core",))
        in_specs = (PartitionSpec("core"),) * (n_params + len(out_names))
        out_specs = (PartitionSpec("core"),) * len(out_names)
        sharded = jax.jit(
            shard_map(_body, mesh=mesh, in_specs=in_specs,
                      out_specs=out_specs, check_rep=False),
            donate_argnums=donate, keep_unused=True)
        ent = (in_names[:n_params], out_names, out_avals, zero_shapes, sharded)
        nc._fast_spmd_ent = ent

    in_names, out_names, out_avals, zero_shapes, sharded = ent

    def _cached_run(nc, in_maps, n_cores):
        concat_in = [
            np.concatenate([np.asarray(m[name]) for m in in_maps], axis=0)
            for name in in_names
        ]
        concat_zeros = [np.zeros((n_cores * s[0], *s[1:]), d)
                        for (s, d) in zero_shapes]
        out_arrs = sharded(*concat_in, *concat_zeros)
        out_np = [np.asarray(o).reshape(n_cores, *out_avals[i].shape)
                  for i, o in enumerate(out_arrs)]
        return [
            {name: out_np[i][c] for i, name in enumerate(out_names)}
            for c in range(n_cores)
        ]

    orig = b2j.run_bass_via_pjrt
    b2j.run_bass_via_pjrt = _cached_run
    try:
        return run_bass_kernel_spmd(nc, in_maps, core_ids=core_ids)
    finally:
        b2j.run_bass_via_pjrt = orig


# ---------------------------------------------------------------- entry point
def _run(inputs, **spmd_kwargs):
    _setup_jax()
    inputs = {k: np.asarray(v) for k, v in inputs.items()}
    h = hashlib.blake2b(digest_size=16)
    for k in sorted(inputs):
        if k != 'x':
            h.update(np.ascontiguousarray(inputs[k]).tobytes())
    wkey = h.hexdigest()
    if wkey not in _CACHE:
        _CACHE[wkey] = _build_program(_prep_weights(inputs))
    nc = _CACHE[wkey]

    x = np.asarray(inputs['x'], dtype=np.float32)   # (32, 1024, 3)
    in_maps = []
    for c in range(NCORES):
        xs = x[c * BPC:(c + 1) * BPC]                       # (4, 1024, 3)
        in_maps.append(
            {'xT': np.ascontiguousarray(xs.transpose(0, 2, 1)).astype(np.float32)})

    if spmd_kwargs:
        from concourse.bass_utils import run_bass_kernel_spmd
        res = run_bass_kernel_spmd(nc, in_maps, core_ids=list(range(NCORES)),
                                   **spmd_kwargs)
    else:
        res = _fast_run_spmd(nc, in_maps, core_ids=list(range(NCORES)))
    out = np.concatenate([r['out'].T for r in res.results], axis=0)  # (32, 40)
    return out.astype(np.float32), res


def kernel(**inputs):
    return _run(inputs)[0]


# revision 9
# speedup vs baseline: 21.7476x; 1.0627x over previous
"""DGCNN forward on 8 Trainium2 NeuronCores, data-parallel over batch.

Contract: kernel(**inputs) takes the FULL (unsharded) inputs from
reference.setup_inputs() and returns the FULL (32, 40) output.

Algorithm (exact, fp32):
  EdgeConv(x)_i = max_{j in knn20(i)} relu(bn(W @ [x_j - x_i; x_i]))
 decomposes (relu/max commute, bn is affine) into
  u_j = s*(wA @ x_j);  v_i = s*((wB-wA) @ x_i) + b
  out_i = relu( max_{j in knn20(i)} u_j  +  v_i )
 so each layer is: pairwise-distance matmul (PE) -> exact top-20 row
 selection (DVE max8/match_replace/max_index) -> gather u rows by index
 (GPSIMD ap_gather) -> windowed max (DVE reduce) -> +v, relu (ACT).

Dispatch: the wall-clock cost of a call is dominated by host/axon
overhead, not device compute, so
  - all weights are baked into the NEFF as Const tensors (DMA'd to HBM
    once at model load); the only runtime input is xT (48 KB/core);
  - the jitted shard_map executable is memoized across calls (the stock
    run_bass_via_pjrt rebuilds + reloads it every call);
  - the jax persistent compilation cache is enabled so a fresh process
    skips the walrus compile.
"""

import hashlib
import numpy as np

B, N, K = 32, 1024, 20
EPS = 1e-5
NCORES = 8
BPC = B // NCORES          # batches per core
NEG = -1e30

_CACHE = {}


def _setup_jax():
    if '_jax' in _CACHE:
        return
    import jax
    jax.config.update("jax_compilation_cache_dir", "/tmp/bass_jax_cache")
    jax.config.update("jax_persistent_cache_min_compile_time_secs", 0.0)
    jax.config.update("jax_persistent_cache_min_entry_size_bytes", 0)
    _CACHE['_jax'] = True


# ---------------------------------------------------------------- weight prep
def _prep_weights(inp):
    """Fold BN into the edge-conv and MLP weights (numpy, host-side)."""
    w = {}
    couts = [64, 64, 64, 128]
    cins = [3, 64, 64, 64]
    for l in range(4):
        wl = inp[f'w{l+1}']            # (Cout, 2C)
        g = inp[f'g{l+1}']
        b = inp[f'b{l+1}']
        C = cins[l]
        s = g / np.sqrt(1.0 + EPS)
        wA = wl[:, :C]                  # acts on (x_j - x_i)
        wB = wl[:, C:]                  # acts on x_i
        Wu = (s[:, None] * wA).T.astype(np.float32)           # (C, Cout)
        Wv = (s[:, None] * (wB - wA)).T.astype(np.float32)    # (C, Cout)
        cout = couts[l]
        if l < 3:
            # batch-pair packing: [Wu | 0] and [0 | Wu], (C, 128)
            zu = np.zeros((C, 64), np.float32)
            w[f'wu{l}a'] = np.concatenate([Wu, zu], 1)
            w[f'wu{l}b'] = np.concatenate([zu, Wu], 1)
            w[f'wv{l}a'] = np.concatenate([Wv, zu], 1)
            w[f'wv{l}b'] = np.concatenate([zu, Wv], 1)
            w[f'bv{l}'] = np.concatenate([b, b]).reshape(128, 1).astype(np.float32)
        else:
            w[f'wu{l}'] = Wu            # (64, 128)
            w[f'wv{l}'] = Wv
            w[f'bv{l}'] = b.reshape(128, 1).astype(np.float32)
    s5 = inp['g5'] / np.sqrt(1.0 + EPS)
    w['w1t'] = (s5[:, None] * inp['lw1']).T.astype(np.float32)      # (320, 1024)
    w['b1'] = (s5 * inp['lb1'] + inp['b5']).reshape(8, 128).T.astype(np.float32).copy()  # (128, 8)
    s6 = inp['g6'] / np.sqrt(1.0 + EPS)
    w['w2t'] = (s6[:, None] * inp['lw2']).T.astype(np.float32)      # (1024, 512)
    w['b2'] = (s6 * inp['lb2'] + inp['b6']).reshape(4, 128).T.astype(np.float32).copy()  # (128, 4)
    w['w3t'] = inp['lw3'].T.astype(np.float32)                      # (512, 40)
    w['b3'] = inp['lb3'].reshape(40, 1).astype(np.float32)
    return w


# ---------------------------------------------------------------- bass program
def _build_program(w):
    """Build the SPMD program with the weights in `w` baked in as NEFF
    constants. Only xT is a runtime input."""
    import concourse.bass as bass
    import concourse.bacc as bacc
    import concourse.mybir as mybir
    from concourse.tile import TileContext

    f32 = mybir.dt.float32
    u16 = mybir.dt.uint16
    i16 = mybir.dt.int16
    AF = mybir.ActivationFunctionType
    AX = mybir.AxisListType

    nc = bacc.Bacc("TRN2")

    # ---- DRAM tensors ----
    xT = nc.dram_tensor("xT", [BPC, 3, N], f32, kind="ExternalInput").ap()
    cins = [3, 64, 64, 64]
    wt = {k: nc.inline_tensor(np.ascontiguousarray(v), name=f"cw_{k}").ap()
          for k, v in w.items()}

    out_d = nc.dram_tensor("out", [40, BPC], f32, kind="ExternalOutput").ap()
    pooled_d = nc.dram_tensor("pooled_stage", [BPC, 320], f32, kind="Internal").ap()

    NPAIR = BPC // 2

    with TileContext(nc) as tc:
        with (
            tc.tile_pool(name="const", bufs=1) as cpool,
            tc.tile_pool(name="wpool", bufs=1) as wpool,
            tc.tile_pool(name="feat", bufs=1) as fpool,
            tc.tile_pool(name="work", bufs=2) as wkpool,
            tc.tile_pool(name="pdp", bufs=6) as pdpool,
            tc.tile_pool(name="sel", bufs=6) as selpool,
            tc.tile_pool(name="gath", bufs=2) as gpool,
            tc.tile_pool(name="ps", bufs=2, space="PSUM") as pspool,
            tc.tile_pool(name="psx", bufs=1, space="PSUM") as psxpool,
        ):
            ones_col = cpool.tile([128, 1], f32, tag="onesc")
            nc.vector.memset(ones_col[:, :], 1.0)
            ones_row = cpool.tile([1, N], f32, tag="onesr")
            nc.vector.memset(ones_row[:, :], 1.0)

            # load weights (all at base partition 0 — the PE requires matmul
            # operands to share a base partition, and mixing tile_positions
            # inside one PSUM accumulation group faults on HW)
            wsb = {}
            for l in range(3):
                for key in (f'wu{l}a', f'wu{l}b', f'wv{l}a', f'wv{l}b'):
                    t = wpool.tile([cins[l], 128], f32, tag=key, name=key)
                    nc.sync.dma_start(t[:, :], wt[key])
                    wsb[key] = t
                t = wpool.tile([128, 1], f32, tag=f'bv{l}', name=f'bv{l}')
                nc.sync.dma_start(t[:, :], wt[f'bv{l}'])
                wsb[f'bv{l}'] = t
            for key in ('wu3', 'wv3'):
                t = wpool.tile([64, 128], f32, tag=key, name=key)
                nc.sync.dma_start(t[:, :], wt[key])
                wsb[key] = t
            t = wpool.tile([128, 1], f32, tag='bv3', name='bv3')
            nc.sync.dma_start(t[:, :], wt['bv3'])
            wsb['bv3'] = t

            # Feature state per pair: paired tile F[p] (128, N) holds unit A
            # in partitions [0:64); FB[p] (64, N) is unit B's copy at base 0
            # (extracted by DMA) so every matmul operand starts at partition 0.
            F = [fpool.tile([128, N], f32, tag=f"F{p}", name=f"F{p}", bufs=2)
                 for p in range(NPAIR)]
            FB = [fpool.tile([64, N], f32, tag=f"FB{p}", name=f"FB{p}", bufs=2)
                  for p in range(NPAIR)]
            for p in range(NPAIR):
                nc.sync.dma_start(F[p][0:3, :], xT[2 * p, :, :])
                nc.sync.dma_start(FB[p][0:3, :], xT[2 * p + 1, :, :])

            for l in range(4):
                C = cins[l]
                for p in range(NPAIR):
                    Fp = F[p]
                    FBp = FB[p]
                    funits = (Fp, FBp)  # unit -> feature AP source (base 0)
                    # ---- squared norms (per unit, base partition 0) ----
                    negxx = [None, None]
                    for ui in range(2):
                        fsq = wkpool.tile([64, N], f32, tag=f"fsq{ui}",
                                          name=f"fsq{ui}")
                        nc.scalar.activation(fsq[0:C, :], funits[ui][0:C, :], AF.Square)
                        xxp = psxpool.tile([1, N], f32, tag="xx", name="xxp")
                        for h in range(2):
                            sl = slice(h * 512, (h + 1) * 512)
                            nc.tensor.matmul(xxp[:, sl], ones_col[0:C, :],
                                             fsq[0:C, sl], start=True, stop=True)
                        nxx = wkpool.tile([1, N], f32, tag=f"nxx{ui}", name=f"nxx{ui}")
                        nc.scalar.activation(nxx[:, :], xxp[:, :], AF.Copy, scale=-1.0)
                        negxx[ui] = nxx

                    # ---- u/v feature tables ----
                    if l < 3:
                        # batch-pair packed: psum = [u_A ; u_B] via padded weights
                        upair = wkpool.tile([128, N], f32, tag="upair")
                        vpair = wkpool.tile([128, N], f32, tag="vpair")
                        for h in range(2):
                            sl = slice(h * 512, (h + 1) * 512)
                            up = pspool.tile([128, 512], f32, tag="acc")
                            vp = pspool.tile([128, 512], f32, tag="acc")
                            nc.tensor.matmul(up[:, :], wsb[f'wu{l}a'][:, :], Fp[0:C, sl],
                                             start=True, stop=False)
                            nc.tensor.matmul(up[:, :], wsb[f'wu{l}b'][:, :],
                                             FBp[0:C, sl], start=False, stop=True)
                            nc.tensor.matmul(vp[:, :], wsb[f'wv{l}a'][:, :], Fp[0:C, sl],
                                             start=True, stop=False)
                            nc.tensor.matmul(vp[:, :], wsb[f'wv{l}b'][:, :],
                                             FBp[0:C, sl], start=False, stop=True)
                            nc.scalar.activation(upair[:, sl], up[:, :], AF.Copy)
                            nc.scalar.activation(vpair[:, sl], vp[:, :], AF.Identity,
                                                 bias=wsb[f'bv{l}'][:, :])
                            del up, vp
                    else:
                        # layer 4: Cout=128 -> per-unit full-width tables
                        u4s, v4s = [], []
                        for ui in range(2):
                            u4 = wkpool.tile([128, N], f32, tag="upair", name=f"u4_{ui}")
                            v4 = wkpool.tile([128, N], f32, tag="vpair", name=f"v4_{ui}")
                            for h in range(2):
                                sl = slice(h * 512, (h + 1) * 512)
                                up = pspool.tile([128, 512], f32, tag="acc")
                                vp = pspool.tile([128, 512], f32, tag="acc")
                                nc.tensor.matmul(up[:, :], wsb['wu3'][:, :],
                                                 funits[ui][0:C, sl], start=True, stop=True)
                                nc.tensor.matmul(vp[:, :], wsb['wv3'][:, :],
                                                 funits[ui][0:C, sl], start=True, stop=True)
                                nc.scalar.activation(u4[:, sl], up[:, :], AF.Copy)
                                nc.scalar.activation(v4[:, sl], vp[:, :], AF.Identity,
                                                     bias=wsb['bv3'][:, :])
                                del up, vp
                            u4s.append(u4)
                            v4s.append(v4)

                    # ---- per-chunk: pd + top-20 + idx pack/broadcast + gather ----
                    # ap_gather reads each core's idx stream from its own 16
                    # partitions in flat order j = c*16 + r; pack point p's 20
                    # idxs to partition p//8, cols (p%8)*20.. so flat j ->
                    # (point 8r + c//20, t = c%20), then window-max reduces the
                    # strided t axis via a 4-D AP. This keeps every DMA on
                    # 40B+ contiguous runs (the old DRAM staging read back
                    # 2-byte elements at 32B stride).
                    def window_max(G, mp_chunk):
                        nc.vector.reduce_max(
                            out=mp_chunk.rearrange("p (r ii) -> p ii r", ii=8),
                            in_=G.rearrange("p (ii t r) -> p ii r t", ii=8, t=K),
                            axis=AX.X)

                    def pd_select(FX, nxxu, isl, wrap_dst):
                        pdp = pspool.tile([128, 1024], f32, tag="pd")
                        for h in range(2):
                            sl = slice(h * 512, (h + 1) * 512)
                            nc.tensor.matmul(pdp[:, sl], FX[0:C, isl],
                                             FX[0:C, sl], start=True, stop=False)
                            nc.tensor.matmul(pdp[:, sl], FX[0:C, isl],
                                             FX[0:C, sl], start=False, stop=False)
                            nc.tensor.matmul(pdp[:, sl], nxxu[:, isl],
                                             ones_row[:, sl], start=False, stop=False)
                            nc.tensor.matmul(pdp[:, sl], ones_row[:, isl],
                                             nxxu[:, sl], start=False, stop=True)
                        pda = pdpool.tile([128, 1024], f32, tag="pda")
                        nc.scalar.activation(pda[:, :], pdp[:, :], AF.Copy)
                        del pdp
                        v0 = selpool.tile([128, 8], f32, tag="v0")
                        v1 = selpool.tile([128, 8], f32, tag="v1")
                        v2 = selpool.tile([128, 8], f32, tag="v2")
                        idx24 = selpool.tile([128, 24], u16, tag="idx24")
                        nc.vector.max(out=v0[:, :], in_=pda[:, :])
                        nc.vector.max_index(out=idx24[:, 0:8], in_max=v0[:, :],
                                            in_values=pda[:, :])
                        pdb = pdpool.tile([128, 1024], f32, tag="pdb")
                        nc.vector.match_replace(out=pdb[:, :], in_to_replace=v0[:, :],
                                                in_values=pda[:, :], imm_value=NEG)
                        nc.vector.max(out=v1[:, :], in_=pdb[:, :])
                        nc.vector.max_index(out=idx24[:, 8:16], in_max=v1[:, :],
                                            in_values=pdb[:, :])
                        nc.vector.match_replace(out=pda[:, :], in_to_replace=v1[:, :],
                                                in_values=pdb[:, :], imm_value=NEG)
                        nc.vector.max(out=v2[:, :], in_=pda[:, :])
                        nc.vector.max_index(out=idx24[:, 16:24], in_max=v2[:, :],
                                            in_values=pda[:, :])
                        wrap, base, nrep = wrap_dst
                        nc.sync.dma_start(
                            wrap[base:base + 16, :].rearrange(
                                "r (ii tt) -> r ii tt", tt=K),
                            idx24[:, 0:K])
                        span = 16
                        while span < nrep * 16:
                            nc.sync.dma_start(wrap[base + span:base + 2 * span, :],
                                              wrap[base:base + span, :])
                            span *= 2

                    if l < 3:
                        Mp = wkpool.tile([128, N], f32, tag="Mp")
                        for ic in range(8):
                            isl = slice(ic * 128, (ic + 1) * 128)
                            wrap = gpool.tile([128, 160], u16, tag="wrap", bufs=3)
                            pd_select(Fp, negxx[0], isl, (wrap, 0, 4))
                            pd_select(FBp, negxx[1], isl, (wrap, 64, 4))
                            G = gpool.tile([128, 2560], f32, tag="G", bufs=3)
                            nc.gpsimd.ap_gather(
                                out_ap=G[:, :], in_ap=upair[:, :],
                                idxs_ap=wrap[:, :].bitcast(i16),
                                channels=128, num_elems=N, d=1, num_idxs=2560)
                            window_max(G, Mp[:, isl])
                        nc.vector.tensor_add(Mp[:, :], Mp[:, :], vpair[:, :])
                        Fnext = fpool.tile([128, N], f32, tag=f"F{p}",
                                           name=f"F{p}_{l}", bufs=2)
                        nc.scalar.activation(Fnext[:, :], Mp[:, :], AF.Relu)
                        FBnext = fpool.tile([64, N], f32, tag=f"FB{p}",
                                            name=f"FB{p}_{l}", bufs=2)
                        nc.sync.dma_start(FBnext[:, :], Fnext[64:128, :])
                        # global max-pool for this layer
                        gp = selpool.tile([128, 1], f32, tag="gp")
                        nc.vector.reduce_max(out=gp[:, :], in_=Fnext[:, :], axis=AX.X)
                        nc.sync.dma_start(pooled_d[2 * p, l * 64:(l + 1) * 64], gp[0:64, :])
                        nc.sync.dma_start(pooled_d[2 * p + 1, l * 64:(l + 1) * 64], gp[64:128, :])
                        F[p], FB[p] = Fnext, FBnext
                    else:
                        for ui in range(2):
                            b = 2 * p + ui
                            Mp = wkpool.tile([128, N], f32, tag="Mp")
                            for ic in range(8):
                                isl = slice(ic * 128, (ic + 1) * 128)
                                wrap = gpool.tile([128, 160], u16, tag="wrap", bufs=3)
                                pd_select(funits[ui], negxx[ui], isl, (wrap, 0, 8))
                                G = gpool.tile([128, 2560], f32, tag="G", bufs=3)
                                nc.gpsimd.ap_gather(
                                    out_ap=G[:, :], in_ap=u4s[ui][:, :],
                                    idxs_ap=wrap[:, :].bitcast(i16),
                                    channels=128, num_elems=N, d=1, num_idxs=2560)
                                window_max(G, Mp[:, isl])
                            nc.vector.tensor_add(Mp[:, :], Mp[:, :], v4s[ui][:, :])
                            x4t = wkpool.tile([128, N], f32, tag="x4t")
                            nc.scalar.activation(x4t[:, :], Mp[:, :], AF.Relu)
                            gp = selpool.tile([128, 1], f32, tag="gp")
                            nc.vector.reduce_max(out=gp[:, :], in_=x4t[:, :], axis=AX.X)
                            nc.sync.dma_start(pooled_d[b, 192:320], gp[:, :])

        # ================= MLP head (own pool scope) =================
        with (
            tc.tile_pool(name="mlp", bufs=1) as mpool,
            tc.tile_pool(name="mps", bufs=2, space="PSUM") as mpspool,
        ):
            pooledT = mpool.tile([128, 3, BPC], f32, tag="pooledT")
            for kc in range(3):
                kn = 128 if kc < 2 else 64
                nc.sync.dma_start(pooledT[0:kn, kc, :],
                                  pooled_d[:, kc * 128:kc * 128 + kn].rearrange("b p -> p b"))
            w1sb = mpool.tile([128, 3, 1024], f32, tag="w1sb")
            for kc in range(3):
                kn = 128 if kc < 2 else 64
                nc.sync.dma_start(w1sb[0:kn, kc, :], wt['w1t'][kc * 128:kc * 128 + kn, :])
            b1sb = mpool.tile([128, 8], f32, tag="b1sb")
            nc.sync.dma_start(b1sb[:, :], wt['b1'])
            h1 = mpool.tile([128, 8, BPC], f32, tag="h1")
            for mc in range(8):
                hp = mpspool.tile([128, BPC], f32, tag="acc")
                for kc in range(3):
                    kn = 128 if kc < 2 else 64
                    nc.tensor.matmul(hp[:, :], w1sb[0:kn, kc, mc * 128:(mc + 1) * 128],
                                     pooledT[0:kn, kc, :], start=(kc == 0), stop=(kc == 2))
                nc.scalar.activation(h1[:, mc, :], hp[:, :], AF.Relu,
                                     bias=b1sb[:, mc:mc + 1])
            w2sb = mpool.tile([128, 8, 512], f32, tag="w2sb")
            for kc in range(8):
                nc.sync.dma_start(w2sb[:, kc, :], wt['w2t'][kc * 128:(kc + 1) * 128, :])
            b2sb = mpool.tile([128, 4], f32, tag="b2sb")
            nc.sync.dma_start(b2sb[:, :], wt['b2'])
            h2 = mpool.tile([128, 4, BPC], f32, tag="h2")
            for mc in range(4):
                hp = mpspool.tile([128, BPC], f32, tag="acc")
                for kc in range(8):
                    nc.tensor.matmul(hp[:, :], w2sb[:, kc, mc * 128:(mc + 1) * 128],
                                     h1[:, kc, :], start=(kc == 0), stop=(kc == 7))
                nc.scalar.activation(h2[:, mc, :], hp[:, :], AF.Relu,
                                     bias=b2sb[:, mc:mc + 1])
            w3sb = mpool.tile([128, 4, 40], f32, tag="w3sb")
            for kc in range(4):
                nc.sync.dma_start(w3sb[:, kc, :], wt['w3t'][kc * 128:(kc + 1) * 128, :])
            b3sb = mpool.tile([40, 1], f32, tag="b3sb")
            nc.sync.dma_start(b3sb[:, :], wt['b3'])
            outp = mpspool.tile([40, BPC], f32, tag="acc")
            for kc in range(4):
                nc.tensor.matmul(outp[:, :], w3sb[:, kc, :], h2[:, kc, :],
                                 start=(kc == 0), stop=(kc == 3))
            outsb = mpool.tile([40, BPC], f32, tag="outsb")
            nc.scalar.activation(outsb[:, :], outp[:, :], AF.Identity, bias=b3sb[:, :])
            nc.sync.dma_start(out_d, outsb[:, :])

    nc.compile()
    # lowering calls nc.to_json_bytes() on every jit retrace; it is pure
    # for a finished program, so memoize it
    jb = nc.to_json_bytes()
    nc.to_json_bytes = lambda: jb
    return nc


# ------------------------------------------------- memoized pjrt dispatch
def _fast_run_spmd(nc, in_maps, core_ids):
    """run_bass_kernel_spmd with the jitted shard_map executable memoized on
    the Bass object (the stock axon path rebuilds jit + reloads the NEFF on
    every call). Temporarily installs a caching run_bass_via_pjrt and goes
    through run_bass_kernel_spmd per the harness contract."""
    import jax
    import concourse.bass2jax as b2j
    import concourse.mybir as mybir
    from concourse.bass_utils import run_bass_kernel_spmd
    from jax.experimental.shard_map import shard_map
    from jax.sharding import Mesh, PartitionSpec

    n_cores = len(core_ids)
    ent = getattr(nc, "_fast_spmd_ent", None)
    if ent is None:
        b2j.install_neuronx_cc_hook()
        assert nc.dbg_addr is None
        partition_name = (nc.partition_id_tensor.name
                          if nc.partition_id_tensor else None)
        in_names, out_names, out_avals, zero_shapes = [], [], [], []
        for alloc in nc.m.functions[0].allocations:
            if not isinstance(alloc, mybir.MemoryLocationSet):
                continue
            name = alloc.memorylocations[0].name
            if alloc.kind == "ExternalInput":
                if name != partition_name:
                    in_names.append(name)
            elif alloc.kind == "ExternalOutput":
                out_names.append(name)
                shape = tuple(alloc.tensor_shape)
                dtype = mybir.dt.np(alloc.dtype)
                out_avals.append(jax.core.ShapedArray(shape, dtype))
                zero_shapes.append((shape, dtype))
        n_params = len(in_names)
        all_names = tuple(in_names + out_names +
                          ([partition_name] if partition_name else []))
        donate = tuple(range(n_params, n_params + len(out_names)))

        def _body(*args):
            operands = list(args)
            if partition_name is not None:
                operands.append(b2j.partition_id_tensor())
            outs = b2j._bass_exec_p.bind(
                *operands,
                out_avals=tuple(out_avals),
                in_names=all_names,
                out_names=tuple(out_names),
                lowering_input_output_aliases=(),
                sim_require_finite=True,
                sim_require_nnan=True,
                nc=nc,
            )
            return tuple(outs)

        devices = jax.devices()[:n_cores]
        assert len(devices) == n_cores
        mesh = Mesh(np.asarray(devices), ("core",))
        in_specs = (PartitionSpec("core"),) * (n_params + len(out_names))
        out_specs = (PartitionSpec("core"),) * len(out_names)
        sharded = jax.jit(
            shard_map(_body, mesh=mesh, in_specs=in_specs,
                      out_specs=out_specs, check_rep=False),
            donate_argnums=donate, keep_unused=True)
        ent = (in_names[:n_params], out_names, out_avals, zero_shapes, sharded)
        nc._fast_spmd_ent = ent

    in_names, out_names, out_avals, zero_shapes, sharded = ent

    def _cached_run(nc, in_maps, n_cores):
        concat_in = [
            np.concatenate([np.asarray(m[name]) for m in in_maps], axis=0)
            for name in in_names
        ]
        concat_zeros = [np.zeros((n_cores * s[0], *s[1:]), d)
                        for (s, d) in zero_shapes]
        out_arrs = sharded(*concat_in, *concat_zeros)
        out_np = [np.asarray(o).reshape(n_cores, *out_avals[i].shape)
                  for i, o in enumerate(out_arrs)]
        return [
            {name: out_np[i][c] for i, name in enumerate(out_names)}
            for c in range(n_cores)
        ]

    orig = b2j.run_bass_via_pjrt
    b2j.run_bass_via_pjrt = _cached_run
    try:
        return run_bass_kernel_spmd(nc, in_maps, core_ids=core_ids)
    finally:
        b2j.run_bass_via_pjrt = orig


# ---------------------------------------------------------------- entry point
def _run(inputs, **spmd_kwargs):
    _setup_jax()
    inputs = {k: np.asarray(v) for k, v in inputs.items()}
    h = hashlib.blake2b(digest_size=16)
    for k in sorted(inputs):
        if k != 'x':
            h.update(np.ascontiguousarray(inputs[k]).tobytes())
    wkey = h.hexdigest()
    if wkey not in _CACHE:
        _CACHE[wkey] = _build_program(_prep_weights(inputs))
    nc = _CACHE[wkey]

    x = np.asarray(inputs['x'], dtype=np.float32)   # (32, 1024, 3)
    in_maps = []
    for c in range(NCORES):
        xs = x[c * BPC:(c + 1) * BPC]                       # (4, 1024, 3)
        in_maps.append(
            {'xT': np.ascontiguousarray(xs.transpose(0, 2, 1)).astype(np.float32)})

    if spmd_kwargs:
        from concourse.bass_utils import run_bass_kernel_spmd
        res = run_bass_kernel_spmd(nc, in_maps, core_ids=list(range(NCORES)),
                                   **spmd_kwargs)
    else:
        res = _fast_run_spmd(nc, in_maps, core_ids=list(range(NCORES)))
    out = np.concatenate([r['out'].T for r in res.results], axis=0)  # (32, 40)
    return out.astype(np.float32), res


def kernel(**inputs):
    return _run(inputs)[0]
